# revision 32
# baseline (speedup 1.0000x reference)
"""Trainium2 Bass kernel for DeformationNetworkGraphConvolutionalFullRes.

Full (unsharded) inputs in, full output out. Data-parallel over the 4 meshes:
core m processes mesh m. Inside each core:

  - vert_align sampling as (S @ F) @ W == S @ (F @ W): per feature map,
    F[C,HW] @ Wslice[C,128] -> G[HW,128] (bf16), then the sparse bilinear
    operator S applied as dense [128px, 512vert] bf16 blocks on the
    TensorEngine (ragged per-chunk schedule), accumulating in PSUM.
    Vertices pre-sorted by image cell.
  - Each GraphConv layer routes its 61440 directed-edge messages through TWO
    independent engines in parallel:
      * DMA half: h1 rows written to HBM in partition-major layout (1KB
        contiguous runs per partition), messages pulled with dma_gather in
        dst-sorted order. Per-tile message counts are padded to the max
        over meshes so the subchunk structure is shared; subchunks are
        packed per scatter GROUP (tile boundaries may fall mid-subchunk),
        each (subchunk, covered-tile) pair scattering with its own
        host-baked dstloc column -- no per-tile ceil padding is gathered.
      * ap half: h1 kept as f32 columns in SBUF (double-banked; 2 source
        windows of 3072 columns each, only sub-window sources are
        ap-routable); gpsimd.ap_gather selects message columns (Pool), PE
        transposes them to row form. Calls are >= the window size so the
        cost is output-bound.
    Both halves are scatter-added per dst tile with one-hot matmuls
    (one is_equal build per group on DVE from a bf16 dstloc table vs an
    iota row; ap one-hots built once, SBUF-resident), accumulating in PSUM
    on top of h0 = W0^T x (+ rank-1 image-encoding term at layer 0); ReLU
    writes the bf16 column-form activations in place. h1 rows/cols for
    layer l+1 are emitted per scatter group of layer l; the double-banked
    h1c removes the end-of-layer flush. Gathers for layer l+1's first
    groups are emitted at the end of layer l so their descriptor
    generation overlaps the boundary. The output projection is fused into
    the last layer's group loop.
"""

import ml_dtypes
import numpy as np
from contextlib import ExitStack

import concourse.bass as bass
import concourse.tile as tile
from concourse import bacc, mybir
from concourse.bass_utils import run_bass_kernel_spmd

# ---------------- problem constants (hardcoded per spec) ----------------
B = 4
V = 10242
E_PER = 30720
HID = 128
MAPS = [(256, 56), (512, 28), (1024, 14), (2048, 7)]  # (C, H==W)
CH_OFF = [0, 256, 768, 1792, 3840]

VP = 10752            # padded vertex count: 84 tiles of 128
NT = VP // 128        # 84 vertex tiles
NVCH = VP // 512      # 21 vertex chunks (sampling)
NW = 2                # ap-gather source windows
W_BASE = [0, 5120]    # window start slots (w0: 10 chunks, w1: 11 chunks)
SUBW = 3072           # ap-routable sub-window (6 chunks of 512)
GT = 8                # dst tiles per scatter group
NGRP = (NT + GT - 1) // GT  # 11 groups (last has 4 tiles)
CALL_SUBS = 32        # max ap-gather call size (subchunks of 128)
AP_FRAC = 0.47        # target fraction of edges through the ap path
PREFETCH = 2          # dma-gather prefetch depth (groups)

F32 = mybir.dt.float32
BF16 = mybir.dt.bfloat16
FP8 = mybir.dt.float8e4
I16 = mybir.dt.int16
AF = mybir.ActivationFunctionType

# sub-window chunks: chunk -> (window, col position in bank)
SUB_CHUNKS = {}
for _w in range(NW):
    for _i in range(SUBW // 512):
        _c = W_BASE[_w] // 512 + _i
        SUB_CHUNKS[_c] = (_w, _w * SUBW + _i * 512)


def _corners(grid, W):
    """grid [V,2] in [-1,1] -> list of (pix_idx int64, weight f32) per corner."""
    x = (grid[:, 0] + 1.0) * 0.5 * (W - 1)
    y = (grid[:, 1] + 1.0) * 0.5 * (W - 1)
    x0f, y0f = np.floor(x), np.floor(y)
    wx1, wy1 = (x - x0f).astype(np.float32), (y - y0f).astype(np.float32)
    wx0, wy0 = 1.0 - wx1, 1.0 - wy1
    x0 = np.clip(x0f, 0, W - 1).astype(np.int64)
    x1 = np.clip(x0f + 1, 0, W - 1).astype(np.int64)
    y0 = np.clip(y0f, 0, W - 1).astype(np.int64)
    y1 = np.clip(y0f + 1, 0, W - 1).astype(np.int64)
    return [
        (y0 * W + x0, wy0 * wx0),
        (y0 * W + x1, wy0 * wx1),
        (y1 * W + x0, wy1 * wx0),
        (y1 * W + x1, wy1 * wx1),
    ]


def _wrap16(idx):
    """int array [n] (n % 16 == 0) -> [128, n/16] wrapped+replicated for the
    8 Q7 cores (idx i at (i%16, i//16))."""
    return np.tile(idx.reshape(-1, 16).T, (8, 1)).astype(np.int16)


def _prep(inputs):
    """Host-side restructuring. Returns (cfg, per_core_aux_list, post)."""
    feats = [inputs["feat1"], inputs["feat2"], inputs["feat3"], inputs["feat4"]]
    av = np.asarray(inputs["aligned_verts"], np.float32)
    verts = np.asarray(inputs["verts_packed"], np.float32)
    enc = np.asarray(inputs["image_enc"], np.float32)
    edges = np.asarray(inputs["edges"], np.int64)

    for bn in ["bottleneck_b", "g0_b0", "g0_b1", "off_b"]:
        assert not np.any(np.asarray(inputs[bn])), f"{bn} nonzero: unsupported"
    assert not np.any(np.asarray(inputs["gb0"])) and not np.any(
        np.asarray(inputs["gb1"])
    ), "gb nonzero: unsupported"

    # per-mesh vertex sort (by finest-map cell): slot = rank in sorted order,
    # pad slots at the very end [V, VP).
    slot_of = []
    corners_all = []
    for m in range(B):
        grid = av[m, :, :2]
        cs = _corners(grid, MAPS[0][1])
        key = cs[0][0]
        sigma = np.argsort(key, kind="stable")
        slot = np.empty(V, np.int64)
        slot[sigma] = np.arange(V)
        slot_of.append(slot)
        corners_all.append([_corners(grid, Wm) for (_, Wm) in MAPS])

    vert_at = []
    for m in range(B):
        va = np.full(VP, -1, np.int64)
        va[slot_of[m][np.arange(V)]] = np.arange(V)
        vert_at.append(va)

    # sampling schedule (ragged: real (map, tile) pairs per chunk, no padding)
    ntile_map = [(Wm * Wm + 127) // 128 for (_, Wm) in MAPS]
    g_off = np.cumsum([0] + ntile_map)
    pairs = []
    for c in range(NVCH):
        lo, hi = c * 512, (c + 1) * 512
        pc = []
        for mi in range(4):
            tiles = set()
            for m in range(B):
                vs = vert_at[m][lo:hi]
                vs = vs[vs >= 0]
                if len(vs):
                    for (pix, _w) in corners_all[m][mi]:
                        tiles.update(np.unique(pix[vs] // 128).tolist())
            for t in sorted(tiles):
                pc.append((mi, t))
        if not pc:
            pc = [(0, 0)]  # zero-weight fallback so PSUM group is well-formed
        pairs.append(pc)
    pair_off = np.concatenate([[0], np.cumsum([len(p) for p in pairs])])
    npair = int(pair_off[-1])
    HNP = max((len(p) + 1) // 2 for p in pairs)

    # graph structure ------------------------------------------------------
    # directed edges in slot space, sorted by (dst tile, src window, src)
    per_mesh_edges = []
    cnt_tw = np.zeros((B, NT, NW), np.int64)      # all msgs per (tile, win)
    cnt_tw_sub = np.zeros((B, NT, NW), np.int64)  # src in sub-window
    for m in range(B):
        e = edges[m * E_PER:(m + 1) * E_PER] - m * V
        a = slot_of[m][e[:, 0]]
        b_ = slot_of[m][e[:, 1]]
        dst = np.concatenate([a, b_])
        src = np.concatenate([b_, a])
        win = (src >= W_BASE[1]).astype(np.int64)
        order = np.lexsort((src, win, dst // 128))
        dst, src, win = dst[order], src[order], win[order]
        per_mesh_edges.append((dst, src, win))
        tl = dst // 128
        insub = (src - np.asarray(W_BASE)[win]) < SUBW
        for t in range(NT):
            sel = tl == t
            for w in range(NW):
                sw = sel & (win == w)
                cnt_tw[m, t, w] = np.sum(sw)
                cnt_tw_sub[m, t, w] = np.sum(sw & insub)

    # ap routing: n_ap[t][w] subchunks through the ap path (shared). Pairs
    # with >= PART_THR leftover sub-window messages get a PARTIAL subchunk
    # (padded with no-op slots) -- cheaper than gathering them over DMA.
    PART_THR = 112
    min_cnt = cnt_tw_sub.min(axis=0)  # [NT, NW]
    n_ap = np.minimum(min_cnt // 128 + (min_cnt % 128 >= PART_THR), 2)
    n_ap = n_ap.astype(np.int64)
    ap_real = np.minimum(128 * n_ap, min_cnt)  # real (non-pad) ap slots
    target_slots = int(AP_FRAC * 2 * E_PER)
    cur = int(ap_real.sum())
    marginal = ap_real - 128 * np.maximum(n_ap - 1, 0)
    order2 = np.argsort(marginal.reshape(-1))
    for idx in order2:
        if cur <= target_slots:
            break
        t, w = divmod(int(idx), NW)
        if n_ap[t, w] > 0:
            n_ap[t, w] -= 1
            nr = int(min(128 * n_ap[t, w], min_cnt[t, w]))
            cur -= int(ap_real[t, w]) - nr
            ap_real[t, w] = nr
    # DMA-half remainder counts, padded to the max over meshes so the
    # subchunk structure (tile spans) is identical on every core. The first
    # SPLIT_GROUPS scatter groups are segmented by source range at SRC_CUT:
    # segment E (src < SRC_CUT) can be gathered before the last h1 writes of
    # the previous layer land; only segment L waits for all of them.
    rcE = np.zeros((B, NT), np.int64)
    rcL = np.zeros((B, NT), np.int64)
    SRC_CUT = (NT - 2 * GT) * 128  # tiles [0, 68): h1 writes 0..16 of 21
    for m in range(B):
        dst, src, win = per_mesh_edges[m]
        pos = 0
        for t in range(NT):
            for w in range(NW):
                c = int(cnt_tw[m, t, w])
                s_ = src[pos:pos + c]
                na = int(ap_real[t, w])
                rem = s_[na:]
                rcE[m, t] += int(np.sum(rem < SRC_CUT))
                rcL[m, t] += int(np.sum(rem >= SRC_CUT))
                pos += c
    rc = rcE + rcL
    assert (rc >= 0).all()
    cnt_pad = rc.max(axis=0)    # [NT] combined (unsegmented groups)
    cnt_padE = rcE.max(axis=0)  # [NT] segmented groups
    cnt_padL = rcL.max(axis=0)
    SPLIT_GROUPS = 1

    def _group_slots(g):
        """Shared per-slot tile array for group g (-1 = pad). Returns
        (tl_slots, cutA) where cutA is the subchunk count of segment E
        (0 for unsegmented groups)."""
        t_lo, t_hi = g * GT, min((g + 1) * GT, NT)
        if g < SPLIT_GROUPS:
            segE = np.concatenate(
                [np.full(int(cnt_padE[t]), t, np.int64)
                 for t in range(t_lo, t_hi)])
            padE = (-len(segE)) % 128
            segL = np.concatenate(
                [np.full(int(cnt_padL[t]), t, np.int64)
                 for t in range(t_lo, t_hi)])
            tl_slots = np.concatenate(
                [segE, np.full(padE, -1, np.int64), segL])
            cutA = (len(segE) + padE) // 128
        else:
            tl_slots = np.concatenate(
                [np.full(int(cnt_pad[t]), t, np.int64)
                 for t in range(t_lo, t_hi)])
            cutA = 0
        padn = (-len(tl_slots)) % 128
        tl_slots = np.concatenate([tl_slots, np.full(padn, -1, np.int64)])
        return tl_slots, cutA

    # group-merged DMA subchunk structure (shared across meshes)
    gsub_off = [0]
    nb_off = [0]
    gmeta = []  # per group: dict(ng, nb, mm=[(j, bloc|None, t_off)], cutA)
    for g in range(NGRP):
        t_lo = g * GT
        tl_slots, cutA = _group_slots(g)
        ng = len(tl_slots) // 128
        mm = []
        nb = 0
        bcols = []
        tk0 = np.full(ng, -1, np.int64)
        for j in range(ng):
            seg = tl_slots[j * 128:(j + 1) * 128]
            tiles_j = sorted(set(int(t) for t in seg if t >= 0))
            if not tiles_j:
                continue
            tk0[j] = tiles_j[0]
            mm.append((j, None, tiles_j[0] - t_lo))
            for t in tiles_j[1:]:
                mm.append((j, nb, t - t_lo))
                bcols.append((j, t))
                nb += 1
        gmeta.append({"ng": ng, "nb": nb, "mm": mm, "bcols": bcols,
                      "tk0": tk0, "cutA": cutA, "tl_slots": tl_slots})
        gsub_off.append(gsub_off[-1] + ng)
        nb_off.append(nb_off[-1] + nb)
    S_dma = gsub_off[-1]
    NB = nb_off[-1]
    MAXSUB_G = max(gm["ng"] for gm in gmeta)
    MAXOH_G = max(gm["ng"] + gm["nb"] for gm in gmeta)

    # ap stream layout per window: subchunk ranges per tile
    ap_off = np.zeros((NT + 1, NW), np.int64)
    for w in range(NW):
        ap_off[1:, w] = np.cumsum(n_ap[:, w])
    S_ap = [int(ap_off[NT, w]) for w in range(NW)]
    S_ap_tot = sum(S_ap)

    # ap gather call partition per stream: whole-GROUP tile ranges, each
    # call <= CALL_SUBS subchunks.
    calls = []  # per w: list of (sub0, sub1, t0, t1, g_start, g_end)
    for w in range(NW):
        cl = []
        g0 = 0
        while g0 < NGRP:
            g1 = g0
            while (g1 < NGRP
                   and ap_off[min((g1 + 1) * GT, NT), w]
                   - ap_off[g0 * GT, w] <= CALL_SUBS):
                g1 += 1
            assert g1 > g0, f"group {g0} stream {w} exceeds CALL_SUBS"
            t0, t1 = g0 * GT, min(g1 * GT, NT)
            s0, s1 = int(ap_off[t0, w]), int(ap_off[t1, w])
            if s1 > s0:
                cl.append((s0, s1, t0, t1, g0, g1 - 1))
            g0 = g1
        calls.append(cl)

    # trmsg rotation safety (shared pool of 3 bufs, merged emission order):
    # tenant i's write must not wait on consumers later than its own readers.
    merged = []
    for w in range(NW):
        for k, c in enumerate(calls[w]):
            merged.append((c[4], c[5], w, k))
    merged.sort()
    for i in range(3, len(merged)):
        assert merged[i][0] > merged[i - 3][1], (
            f"trmsg rotation hazard: {merged[i]} vs {merged[i-3]}")

    S_tot = S_dma + NB + S_ap_tot
    MAXCALL = max(max((c[1] - c[0]) for c in cl) for cl in calls if cl)

    cfg = {"MAXCALL": MAXCALL, "pairs": pairs, "HNP": HNP,
           "pair_off": pair_off.tolist(), "g_off": g_off.tolist(),
           "ntile_map": ntile_map,
           "gsub_off": gsub_off, "gmeta": gmeta, "nb_off": nb_off,
           "S_dma": S_dma, "NB": NB, "MAXSUB_G": MAXSUB_G, "MAXOH_G": MAXOH_G,
           "n_ap": n_ap.tolist(), "ap_off": ap_off.tolist(),
           "S_ap": S_ap, "calls": calls, "S_tot": S_tot, "npair": npair}

    # ---------------- per-core tables ----------------
    per_core = []
    for m in range(B):
        dst, src, win = per_mesh_edges[m]
        ap_idx = [np.zeros(max(16, S_ap[w] * 128), np.int64) for w in range(NW)]
        ap_dl = [np.full(S_ap[w] * 128, -(10 ** 6), np.int64)
                 for w in range(NW)]
        rem_d = [[] for _ in range(NT)]
        rem_s = [[] for _ in range(NT)]
        pos = 0
        for t in range(NT):
            for w in range(NW):
                c = int(cnt_tw[m, t, w])
                d_, s_ = dst[pos:pos + c], src[pos:pos + c]
                na = int(ap_real[t, w])
                o = int(ap_off[t, w]) * 128
                # msgs sorted by src within (t, w): the first na are in the
                # sub-window (na <= cnt_tw_sub[m,t,w] by construction)
                ap_idx[w][o:o + na] = s_[:na] - W_BASE[w]
                ap_dl[w][o:o + na] = d_[:na] - t * 128
                rem_d[t].append(d_[na:])
                rem_s[t].append(s_[na:])
                pos += c
        assert pos == 2 * E_PER
        for w in range(NW):
            assert (ap_idx[w] >= 0).all() and (ap_idx[w] < SUBW).all()

        # group-merged DMA streams (per-tile pad to the shared counts; split
        # groups are segmented by SRC_CUT with segment E padded to a
        # subchunk boundary)
        src_slots = np.zeros(max(16, S_dma * 128), np.int64)
        dl_gr = np.full((S_dma + NB) * 128, -1, np.int64)

        def _tile_seg(t, sel_fn, padto):
            d_ = np.concatenate(rem_d[t])
            s_ = np.concatenate(rem_s[t])
            sel = sel_fn(s_)
            d_, s_ = d_[sel], s_[sel]
            padn = int(padto) - len(d_)
            assert padn >= 0
            return (np.concatenate([d_, np.full(padn, -1, np.int64)]),
                    np.concatenate([s_, np.zeros(padn, np.int64)]))

        for g in range(NGRP):
            t_lo, t_hi = g * GT, min((g + 1) * GT, NT)
            gm = gmeta[g]
            ds, ss = [], []
            if gm["cutA"]:
                for t in range(t_lo, t_hi):
                    d2, s2 = _tile_seg(t, lambda s: s < SRC_CUT, cnt_padE[t])
                    ds.append(d2)
                    ss.append(s2)
                padE = gm["cutA"] * 128 - sum(len(x) for x in ds)
                assert padE >= 0
                ds.append(np.full(padE, -1, np.int64))
                ss.append(np.zeros(padE, np.int64))
                for t in range(t_lo, t_hi):
                    d2, s2 = _tile_seg(t, lambda s: s >= SRC_CUT, cnt_padL[t])
                    ds.append(d2)
                    ss.append(s2)
            else:
                for t in range(t_lo, t_hi):
                    d2, s2 = _tile_seg(t, lambda s: np.ones(len(s), bool),
                                       cnt_pad[t])
                    ds.append(d2)
                    ss.append(s2)
            d_ = np.concatenate(ds) if ds else np.zeros(0, np.int64)
            s_ = np.concatenate(ss) if ss else np.zeros(0, np.int64)
            n = len(d_)
            ng, nb = gm["ng"], gm["nb"]
            so = gsub_off[g] * 128
            src_slots[so:so + n] = s_
            base2 = (gsub_off[g] + nb_off[g]) * 128
            jj = np.arange(n) // 128
            tk0 = np.maximum(np.asarray(gm["tk0"], np.int64), 0)
            dv = np.where(d_ >= 0, d_ - 128 * tk0[jj], -(10 ** 6))
            dl_gr[base2:base2 + n] = dv
            for bi, (j, t) in enumerate(gm["bcols"]):
                col = base2 + (ng + bi) * 128
                lo_s, hi_s = j * 128, min(j * 128 + 128, n)
                seg = d_[lo_s:hi_s]
                dl_gr[col:col + hi_s - lo_s] = np.where(
                    seg >= 0, seg - 128 * t, -(10 ** 6))
        # clamp for bf16 safety: anything outside [0,128) just must not
        # collide with iota values after rounding; keep magnitudes small.
        dl_gr = np.clip(dl_gr, -512, 1024)
        # partition-major h1d row mapping
        rows = (src_slots % 128) * NT + src_slots // 128

        dl_all = np.concatenate([dl_gr] + ap_dl)
        dl_tab = dl_all.reshape(S_tot, 128).T.copy().astype(ml_dtypes.bfloat16)

        srcw = _wrap16(rows)
        apw = [_wrap16(ap_idx[w]) for w in range(NW)]

        # sampling blocks ---------------------------------------------------
        wsc = np.zeros((npair, 128, 512), np.float32)
        pi = 0
        for c in range(NVCH):
            lo = c * 512
            vs_all = vert_at[m][lo:lo + 512]
            jj2 = np.nonzero(vs_all >= 0)[0]
            for (mi, t) in pairs[c]:
                blk = wsc[pi]
                if len(jj2):
                    for (pix, w_) in corners_all[m][mi]:
                        px = pix[vs_all[jj2]]
                        sel = (px >= t * 128) & (px < (t + 1) * 128)
                        j3 = jj2[sel]
                        np.add.at(blk, (pix[vs_all[j3]] - t * 128, j3),
                                  w_[vs_all[j3]])
                pi += 1
        assert pi == npair

        vt = np.zeros((3, VP), np.float32)
        vslots = slot_of[m][np.arange(V)]
        vt[:, vslots] = verts[m * V:(m + 1) * V].T

        bf = ml_dtypes.bfloat16
        aux = {
            "f1": feats[0][m].reshape(256, -1).astype(bf),
            "f2": feats[1][m].reshape(512, -1).astype(bf),
            "f3": feats[2][m].reshape(1024, -1).astype(bf),
            "f4": feats[3][m].reshape(2048, -1).astype(bf),
            "bw": np.asarray(inputs["bottleneck_w"], np.float32).astype(bf),
            "wsc": wsc.reshape(npair * 128, 512).astype(bf),
            "srcw": np.ascontiguousarray(srcw),
            "apw0": np.ascontiguousarray(apw[0]),
            "apw1": np.ascontiguousarray(apw[1]),
            "dstloc": np.ascontiguousarray(dl_tab),
            "iota": np.tile(np.arange(128, dtype=bf), (128, 1)),
            "ident": np.eye(128, dtype=np.float32),
            "vertsT": vt.astype(bf),
            "encc": enc[m].reshape(2, 128).T.copy(),
            "g0w0m": np.asarray(inputs["g0_w0"][:128], np.float32).astype(bf),
            "g0w1m": np.asarray(inputs["g0_w1"][:128], np.float32).astype(bf),
            "g0w0v": np.asarray(inputs["g0_w0"][128:131], np.float32).astype(bf),
            "g0w1v": np.asarray(inputs["g0_w1"][128:131], np.float32).astype(bf),
            "g0w0e": np.ascontiguousarray(
                np.asarray(inputs["g0_w0"][131:387], np.float32)),
            "g0w1e": np.ascontiguousarray(
                np.asarray(inputs["g0_w1"][131:387], np.float32)),
            "gw0": np.ascontiguousarray(
                np.asarray(inputs["gw0"], np.float32).transpose(1, 0, 2)
                .reshape(128, 7 * 128)).astype(bf),
            "gw1": np.ascontiguousarray(
                np.asarray(inputs["gw1"], np.float32).transpose(1, 0, 2)
                .reshape(128, 7 * 128)).astype(bf),
            "offw": np.asarray(inputs["off_w"], np.float32).astype(bf),
        }
        per_core.append(aux)

    post = {"slot_of": slot_of}
    return cfg, per_core, post


def _build(cfg, shapes, nlayers=8, repeat=1):
    """Build the SPMD Bass program (same instruction stream for all cores)."""
    nc = bacc.Bacc("TRN2", target_bir_lowering=False, debug=False, num_devices=B)
    ap = {}
    for name, arr in shapes.items():
        ap[name] = nc.dram_tensor(
            name, list(arr.shape), mybir.dt.from_np(arr.dtype),
            kind="ExternalInput").ap()
    out = nc.dram_tensor("out", [VP, 3], F32, kind="ExternalOutput").ap()
    h1d2 = [nc.dram_tensor("h1da", [VP, HID], BF16).ap(),
            nc.dram_tensor("h1db", [VP, HID], BF16).ap()]

    pairs = cfg["pairs"]
    pair_off = cfg["pair_off"]
    HNP = cfg["HNP"]
    g_off = cfg["g_off"]
    ntile_map = cfg["ntile_map"]
    NGT_ = g_off[4]
    gsub_off = cfg["gsub_off"]
    gmeta = cfg["gmeta"]
    nb_off = cfg["nb_off"]
    S_dma = cfg["S_dma"]
    NB = cfg["NB"]
    MAXSUB_G = cfg["MAXSUB_G"]
    MAXOH_G = cfg["MAXOH_G"]
    n_ap = cfg["n_ap"]
    ap_off = cfg["ap_off"]
    S_ap = cfg["S_ap"]
    calls = cfg["calls"]
    S_tot = cfg["S_tot"]
    S_ap_tot = sum(S_ap)
    MAXCALL = cfg["MAXCALL"]
    ap_base = [S_dma + NB, S_dma + NB + S_ap[0]]

    chunks_by_group = {}
    for c in range(NVCH):
        g = (4 * c + 3) // GT
        chunks_by_group.setdefault(g, []).append(c)

    with tile.TileContext(nc) as tc, ExitStack() as ctx:
        # ---------------- persistent pool ----------------
        pp = ctx.enter_context(tc.tile_pool(name="pers", bufs=1))
        xx = pp.tile([128, VP], BF16, tag="xx")
        h1c0 = pp.tile([128, NW * SUBW], F32, tag="h1c0")
        h1c1 = pp.tile([128, NW * SUBW], F32, tag="h1c1")
        h1c_banks = [h1c0, h1c1]
        oh_ap = pp.tile([128, max(1, S_ap_tot), 128], FP8, tag="ohap")
        srcw_t = pp.tile([128, max(1, S_dma) * 8], I16, tag="srcw")
        apw0_t = pp.tile([128, max(1, S_ap[0]) * 8], I16, tag="apw0")
        apw1_t = pp.tile([128, max(1, S_ap[1]) * 8], I16, tag="apw1")
        apw_t = [apw0_t, apw1_t]
        dstloc_t = pp.tile([128, S_tot, 1], BF16, tag="dstloc")
        iota_t = pp.tile([128, 1, 128], BF16, tag="iota")
        ident_t = pp.tile([128, 128], F32, tag="ident")
        w0_t = pp.tile([128, 7 * 128], BF16, tag="w0")
        w1_t = pp.tile([128, 7 * 128], BF16, tag="w1")
        g0m_t = pp.tile([128, 2 * 128], BF16, tag="g0m")
        g0v_t = pp.tile([3, 256], BF16, tag="g0v")
        offw_t = pp.tile([128, 3], BF16, tag="offw")
        ones_t = pp.tile([1, 512], BF16, tag="ones")
        erow_t = pp.tile([1, 256], BF16, tag="erow")
        encc_t = pp.tile([128, 2], F32, tag="encc")

        nc.sync.dma_start(srcw_t[:], ap["srcw"][:])
        for w in range(NW):
            nc.sync.dma_start(apw_t[w][:], ap[f"apw{w}"][:])
        nc.sync.dma_start(
            dstloc_t[:], ap["dstloc"].rearrange("p (s o) -> p s o", o=1))
        nc.sync.dma_start(iota_t[:].rearrange("p o d -> p (o d)"), ap["iota"][:])
        nc.sync.dma_start(ident_t[:], ap["ident"][:])
        nc.sync.dma_start(w0_t[:], ap["gw0"][:])
        nc.sync.dma_start(w1_t[:], ap["gw1"][:])
        nc.sync.dma_start(g0m_t[:, 0:128], ap["g0w0m"][:])
        nc.sync.dma_start(g0m_t[:, 128:256], ap["g0w1m"][:])
        nc.sync.dma_start(g0v_t[:, 0:128], ap["g0w0v"][:])
        nc.sync.dma_start(g0v_t[:, 128:256], ap["g0w1v"][:])
        nc.sync.dma_start(offw_t[:], ap["offw"][:])
        nc.vector.memset(ones_t[:], 1.0)
        nc.sync.dma_start(encc_t[:], ap["encc"][:])

        # ap one-hots, built once (fp8, resident)
        if S_ap_tot:
            nc.vector.tensor_tensor(
                out=oh_ap[:, :S_ap_tot, :],
                in0=dstloc_t[:, S_dma + NB:S_tot, :]
                .to_broadcast([128, S_ap_tot, 128]),
                in1=iota_t[:].to_broadcast([128, S_ap_tot, 128]),
                op=mybir.AluOpType.is_equal)

        lph = ctx.enter_context(tc.tile_pool(name="hst", bufs=2))
        lpv = ctx.enter_context(tc.tile_pool(name="vv", bufs=1))

        def emit_h1_rows(l, c0, nt4, h1_writes, pool):
            """h1 rows for layer l, tiles [c0, c0+nt4) -> h1d2[l % 2]
            (partition-major: vertex t*128+p lands at row p*NT+t)."""
            h1d = h1d2[l % 2]
            ph = pool.tile([128, 512], F32, tag="ph")
            if l == 0:
                vv = lpv.tile([3, 512], BF16, tag="vt")
                nc.sync.dma_start(
                    vv[:, :nt4 * 128],
                    ap["vertsT"][:, c0 * 128:(c0 + nt4) * 128])
            for ti in range(nt4):
                t = c0 + ti
                sl = slice(ti * 128, (ti + 1) * 128)
                if l == 0:
                    nc.tensor.matmul(
                        out=ph[:, sl], lhsT=xx[:, t * 128:(t + 1) * 128],
                        rhs=g0m_t[:, 128:256], start=True, stop=False)
                    nc.tensor.matmul(
                        out=ph[:, sl], lhsT=vv[:, ti * 128:(ti + 1) * 128],
                        rhs=g0v_t[:, 128:256], start=False, stop=False)
                    nc.tensor.matmul(
                        out=ph[:, sl], lhsT=ones_t[:, 0:128],
                        rhs=erow_t[:, 128:256], start=False, stop=True)
                else:
                    nc.tensor.matmul(
                        out=ph[:, sl], lhsT=xx[:, t * 128:(t + 1) * 128],
                        rhs=w1_t[:, (l - 1) * 128:l * 128],
                        start=True, stop=True)
            hst = lph.tile([128, 512], BF16, tag="hst")
            nc.scalar.activation(hst[:, :nt4 * 128], ph[:, :nt4 * 128],
                                 AF.Copy)
            h1_writes.append(nc.sync.dma_start(
                h1d.rearrange("(p n) c -> p n c", p=128)[:, c0:c0 + nt4, :],
                hst[:, :nt4 * 128].rearrange("p (n c) -> p n c", c=128)))

        def emit_h1_cols(l, c, pool):
            """h1 column chunk c for layer l -> its h1c bank (sub-window
            chunks only)."""
            if c not in SUB_CHUNKS:
                return
            _w, pos = SUB_CHUNKS[c]
            bank = h1c_banks[l % 2]
            c0 = c * 512
            cw = 512
            ph = pool.tile([128, 512], F32, tag="ph")
            if l == 0:
                vv = lpv.tile([3, 512], BF16, tag="vt")
                nc.sync.dma_start(vv[:, :cw], ap["vertsT"][:, c0:c0 + cw])
                nc.tensor.matmul(
                    out=ph[:], lhsT=g0m_t[:, 128:256],
                    rhs=xx[:, c0:c0 + cw], start=True, stop=False)
                nc.tensor.matmul(
                    out=ph[:], lhsT=g0v_t[:, 128:256],
                    rhs=vv[:, :cw], start=False, stop=False)
                nc.tensor.matmul(
                    out=ph[:], lhsT=erow_t[:, 128:256],
                    rhs=ones_t[:, :cw], start=False, stop=True)
            else:
                nc.tensor.matmul(
                    out=ph[:], lhsT=w1_t[:, (l - 1) * 128:l * 128],
                    rhs=xx[:, c0:c0 + cw], start=True, stop=True)
            nc.scalar.activation(bank[:, pos:pos + cw], ph[:], AF.Copy)

        samp_done = []
        h1w0 = []
        with ExitStack() as sctx:
            # ---------------- phase 1: sampling ----------------
            sp = sctx.enter_context(tc.tile_pool(name="samp", bufs=1))
            spf = sctx.enter_context(tc.tile_pool(name="sampf", bufs=3))
            spw = sctx.enter_context(tc.tile_pool(name="sampw", bufs=4))
            spp = sctx.enter_context(
                tc.tile_pool(name="sampps", bufs=2, space="PSUM"))
            spp2 = sctx.enter_context(
                tc.tile_pool(name="sampps2", bufs=2, space="PSUM"))

            g0e_t = sp.tile([128, 4 * 128], F32, tag="g0e")
            nc.sync.dma_start(
                g0e_t[:, 0:256].rearrange("p (c h) -> p c h", h=128),
                ap["g0w0e"].rearrange("(c p) h -> p c h", p=128))
            nc.sync.dma_start(
                g0e_t[:, 256:512].rearrange("p (c h) -> p c h", h=128),
                ap["g0w1e"].rearrange("(c p) h -> p c h", p=128))
            for k in range(2):
                pe = spp2.tile([1, 128], F32, tag="pe")
                for cchunk in range(2):
                    nc.tensor.matmul(
                        out=pe[:],
                        lhsT=encc_t[:, cchunk:cchunk + 1],
                        rhs=g0e_t[:, k * 256 + cchunk * 128:
                                  k * 256 + cchunk * 128 + 128],
                        start=(cchunk == 0), stop=(cchunk == 1))
                nc.scalar.activation(erow_t[:, k * 128:(k + 1) * 128], pe[:],
                                     AF.Copy)

            g_sb = sp.tile([128, NGT_ * 128], BF16, tag="gsb")
            for mi, (C, Wm) in enumerate(MAPS):
                HW = Wm * Wm
                ncc = C // 128
                bw_t = spf.tile([128, 16 * 128], BF16, tag="bw")
                nc.sync.dma_start(
                    bw_t[:, :ncc * 128].rearrange("p (c h) -> p c h", h=128),
                    ap["bw"].rearrange("(c p) h -> p c h", p=128)
                    [:, CH_OFF[mi] // 128:CH_OFF[mi] // 128 + ncc, :])
                fm_t = sp.tile([128, 2 * 3136], BF16, tag="fm")
                nc.sync.dma_start(
                    fm_t[:, :ncc * HW].rearrange("p (c hw) -> p c hw", c=ncc),
                    ap[f"f{mi+1}"].rearrange("(c p) hw -> p c hw", p=128))
                for t in range(ntile_map[mi]):
                    p0 = t * 128
                    pcnt = min(128, HW - p0)
                    pg = spp2.tile([128, 128], F32, tag="pg")
                    for cc in range(ncc):
                        nc.tensor.matmul(
                            out=pg[:pcnt, :],
                            lhsT=fm_t[:, cc * HW + p0:cc * HW + p0 + pcnt],
                            rhs=bw_t[:, cc * 128:cc * 128 + 128],
                            start=(cc == 0), stop=(cc == ncc - 1))
                    gt = g_off[mi] + t
                    nc.scalar.activation(
                        g_sb[:pcnt, gt * 128:gt * 128 + 128], pg[:pcnt, :],
                        AF.Copy)

            for c in range(NVCH):
                ps = spp.tile([128, 512], F32, tag="ps")
                pairs_c = pairs[c]
                npc_c = len(pairs_c)
                half = (npc_c + 1) // 2
                wts = []
                for hb in range(2):
                    k0, k1 = hb * half, min((hb + 1) * half, npc_c)
                    wt = spw.tile([128, HNP, 512], BF16, tag="wsc")
                    if k1 > k0:
                        nc.sync.dma_start(
                            wt[:, :k1 - k0, :],
                            ap["wsc"].rearrange("(k p) h -> p k h", p=128)
                            [:, pair_off[c] + k0:pair_off[c] + k1, :])
                    wts.append(wt)
                for k, (mi, t) in enumerate(pairs_c):
                    HW = MAPS[mi][1] ** 2
                    pcnt = min(128, HW - t * 128)
                    gt = g_off[mi] + t
                    nc.tensor.matmul(
                        out=ps[:],
                        lhsT=g_sb[:pcnt, gt * 128:gt * 128 + 128],
                        rhs=wts[k // half][:pcnt, k % half, :],
                        start=(k == 0), stop=(k == npc_c - 1))
                nc.scalar.activation(xx[:, c * 512:(c + 1) * 512], ps[:],
                                     AF.Relu)
                emit_h1_rows(0, c * 4, 4, h1w0, spp)
                emit_h1_cols(0, c, spp)
                samp_done.append(c)

        # ---------------- phase 2: graph conv layers ----------------
        lp = ctx.enter_context(tc.tile_pool(name="msg", bufs=3))
        apb = ctx.enter_context(tc.tile_pool(name="apbuf", bufs=2))
        trp = ctx.enter_context(tc.tile_pool(name="trmsg", bufs=3))
        ohd = ctx.enter_context(tc.tile_pool(name="ohdma", bufs=2))
        psh = ctx.enter_context(tc.tile_pool(name="psh", bufs=2, space="PSUM"))
        pst = ctx.enter_context(tc.tile_pool(name="pst", bufs=2, space="PSUM"))
        psx = ctx.enter_context(tc.tile_pool(name="psx", bufs=2, space="PSUM"))

        pending = {}    # (l, g) -> (msg, ohg) or None
        pending_b = {}  # (l, g) -> (msg, s0, cutA, ng): deferred B segment

        H1W_CUT = (NT - 2 * GT) // 4  # h1 row writes covering tiles < SRC_CUT

        def _emit_gather_part(l, msg, s0, a, b, deps):
            gi = nc.gpsimd.dma_gather(
                out_ap=msg[:, a:b, :],
                in_ap=h1d2[l % 2][:],
                idxs_ap=srcw_t[:, (s0 + a) * 8:(s0 + b) * 8],
                num_idxs=(b - a) * 128,
                num_idxs_reg=(b - a) * 128,
                elem_size=HID,
                single_packet=False,
            )
            for wi in deps:
                tile.add_dep_helper(gi.ins, wi.ins,
                                    reason="h1 RAW: gather after write")

        def emit_gather(l, g, h1_writes, defer_b=False):
            """dma-gather + one-hot build for (layer l, group g). Split
            groups gather segment E (early sources) with a dependency on
            only the first H1W_CUT h1 writes; with defer_b the late-source
            segment is emitted later via emit_deferred_b."""
            s0, s1 = gsub_off[g], gsub_off[g + 1]
            ng = s1 - s0
            if ng == 0:
                pending[(l, g)] = None
                return
            msg = lp.tile([128, MAXSUB_G, 128], BF16, tag="msg")
            cutA = gmeta[g]["cutA"]
            if 0 < cutA < ng:
                _emit_gather_part(l, msg, s0, 0, cutA, h1_writes[:H1W_CUT])
                if defer_b:
                    pending_b[(l, g)] = (msg, s0, cutA, ng)
                else:
                    _emit_gather_part(l, msg, s0, cutA, ng, h1_writes)
            else:
                _emit_gather_part(l, msg, s0, 0, ng, h1_writes)
            noh = ng + gmeta[g]["nb"]
            b2 = gsub_off[g] + nb_off[g]
            ohg = ohd.tile([128, MAXOH_G, 128], FP8, tag="ohg")
            nc.vector.tensor_tensor(
                out=ohg[:, :noh, :],
                in0=dstloc_t[:, b2:b2 + noh, :].to_broadcast([128, noh, 128]),
                in1=iota_t[:].to_broadcast([128, noh, 128]),
                op=mybir.AluOpType.is_equal)
            pending[(l, g)] = (msg, ohg)

        ap_state = {}  # l -> per-layer ap-call emission state

        def _get_ap_state(l):
            if l not in ap_state:
                ap_state[l] = {
                    "next": [0] * NW,
                    "tr": [[None] * len(calls[w]) for w in range(NW)],
                    "flip": [0],
                }
            return ap_state[l]

        def emit_ap_call(l, w, k):
            st = _get_ap_state(l)
            bank = h1c_banks[l % 2]
            s0, s1, _t0, _t1, _gs, _ge = calls[w][k]
            ns = s1 - s0
            buf = apb.tile([128, MAXCALL * 128], F32, tag="apbuf")
            nc.gpsimd.ap_gather(
                out_ap=buf[:, :ns * 128],
                in_ap=bank[:, w * SUBW:(w + 1) * SUBW],
                idxs_ap=apw_t[w][:, s0 * 8:s1 * 8],
                channels=128, num_elems=SUBW, d=1, num_idxs=ns * 128)
            tr = trp.tile([128, MAXCALL, 128], BF16, tag="trmsg")
            st["tr"][w][k] = (tr, s0)
            for j4 in range(0, ns, 4):
                jn = min(4, ns - j4)
                pt = pst.tile([128, 512], F32, tag="pt")
                for j in range(jn):
                    nc.tensor.transpose(
                        pt[:, j * 128:(j + 1) * 128],
                        buf[:, (j4 + j) * 128:(j4 + j + 1) * 128],
                        ident_t[:])
                dst_sl = tr[:, j4:j4 + jn, :].rearrange("p s o -> p (s o)")
                if st["flip"][0] % 2 == 0:
                    nc.vector.tensor_copy(dst_sl, pt[:, :jn * 128])
                else:
                    nc.scalar.activation(dst_sl, pt[:, :jn * 128], AF.Copy)
                st["flip"][0] += 1

        def emit_eligible_calls(l, g):
            # round-robin across windows so group g's trmsg transposes
            # come before deeper-lookahead calls on the in-order engines
            st = _get_ap_state(l)
            while True:
                did = False
                for w in range(NW):
                    if (st["next"][w] < len(calls[w])
                            and calls[w][st["next"][w]][4] <= g + 2):
                        emit_ap_call(l, w, st["next"][w])
                        st["next"][w] += 1
                        did = True
                if not did:
                    break

        def _layer(l, h1_writes, last_layer):
            """Scatter groups for layer l; h1 for layer l+1 is emitted inside
            (pipelined). Returns layer l+1's h1_writes list."""
            h1_writes_next = []
            trmsg_tiles = _get_ap_state(l)["tr"]

            if (l, 0) in pending_b:
                msg_, s0_, cutA_, ng_ = pending_b.pop((l, 0))
                _emit_gather_part(l, msg_, s0_, cutA_, ng_, h1_writes)
            for g in range(min(PREFETCH, NGRP)):
                if (l, g) not in pending:
                    emit_gather(l, g, h1_writes)

            for g in range(NGRP):
                t_lo = g * GT
                t_hi = min((g + 1) * GT, NT)
                emit_eligible_calls(l, g)
                if g + PREFETCH < NGRP:
                    emit_gather(l, g + PREFETCH, h1_writes)

                W_ = (t_hi - t_lo) * 128
                px = psx.tile([128, GT * 128], F32, tag="px")

                got = pending.pop((l, g))
                if got is not None:
                    msg, ohg = got

                mms = []  # entries: (seg_id, kwargs)
                if l == 0:
                    vv2 = lpv.tile([3, GT * 128], BF16, tag="vt2")
                    nc.sync.dma_start(
                        vv2[:, :W_], ap["vertsT"][:, t_lo * 128:t_hi * 128])
                    for seg in range(0, W_, 512):
                        sw = min(512, W_ - seg)
                        c0 = t_lo * 128 + seg
                        mms.append((seg // 512,
                                    dict(out=px[:, seg:seg + sw],
                                         lhsT=g0m_t[:, 0:128],
                                         rhs=xx[:, c0:c0 + sw])))
                        mms.append((seg // 512,
                                    dict(out=px[:, seg:seg + sw],
                                         lhsT=g0v_t[:, 0:128],
                                         rhs=vv2[:, seg:seg + sw])))
                        mms.append((seg // 512,
                                    dict(out=px[:, seg:seg + sw],
                                         lhsT=erow_t[:, 0:128],
                                         rhs=ones_t[:, :sw])))
                else:
                    for seg in range(0, W_, 512):
                        sw = min(512, W_ - seg)
                        c0 = t_lo * 128 + seg
                        mms.append((seg // 512,
                                    dict(out=px[:, seg:seg + sw],
                                         lhsT=w0_t[:, (l - 1) * 128:l * 128],
                                         rhs=xx[:, c0:c0 + sw])))
                # DMA-half scatter (merged subchunks; one oh tile holds the
                # k0 columns [0, ng) and boundary columns [ng, ng+nb))
                if got is not None:
                    ng = gmeta[g]["ng"]
                    for (j, bloc, t_off) in gmeta[g]["mm"]:
                        ohc = j if bloc is None else ng + bloc
                        osl = slice(t_off * 128, (t_off + 1) * 128)
                        mms.append((t_off * 128 // 512,
                                    dict(out=px[:, osl], lhsT=msg[:, j, :],
                                         rhs=ohg[:, ohc, :])))
                # ap-half scatter
                for ti in range(t_hi - t_lo):
                    t = t_lo + ti
                    osl = slice(ti * 128, (ti + 1) * 128)
                    for w in range(NW):
                        na = n_ap[t][w]
                        if na == 0:
                            continue
                        kk = next(
                            i for i, c in enumerate(calls[w])
                            if c[2] <= t < c[3])
                        tr, trs0 = trmsg_tiles[w][kk]
                        for j in range(na):
                            s_loc = ap_off[t][w] - trs0 + j
                            s_ap = ap_off[t][w] + j + (0 if w == 0 else S_ap[0])
                            mms.append((ti * 128 // 512,
                                        dict(out=px[:, osl],
                                             lhsT=tr[:, s_loc, :],
                                             rhs=oh_ap[:, s_ap, :])))
                first_of = {}
                last_of = {}
                for i, (sg, _kw) in enumerate(mms):
                    first_of.setdefault(sg, i)
                    last_of[sg] = i
                for i, (sg, kw) in enumerate(mms):
                    nc.tensor.matmul(start=(first_of[sg] == i),
                                     stop=(last_of[sg] == i),
                                     skip_group_check=True, **kw)
                nc.scalar.activation(xx[:, t_lo * 128:t_hi * 128], px[:, :W_],
                                     AF.Relu)

                # ---- pipelined layer-(l+1) h1 production ----
                if not last_layer:
                    for c0 in range(t_lo, t_hi, 4):
                        emit_h1_rows(l + 1, c0, min(4, t_hi - c0),
                                     h1_writes_next, psh)
                    for c in chunks_by_group.get(g, []):
                        emit_h1_cols(l + 1, c, psh)
                else:
                    ost = lph.tile([128, GT * 3], F32, tag="ost")
                    for ti in range(t_hi - t_lo):
                        t = t_lo + ti
                        po = psh.tile([128, 512], F32, tag="ph")
                        nc.tensor.matmul(out=po[:, :3],
                                         lhsT=xx[:, t * 128:(t + 1) * 128],
                                         rhs=offw_t[:], start=True, stop=True)
                        nc.scalar.activation(ost[:, ti * 3:(ti + 1) * 3],
                                             po[:, :3], AF.Copy)
                    nc.sync.dma_start(
                        out.rearrange("(n p) c -> p n c", p=128)
                        [:, t_lo:t_hi, :],
                        ost[:, :(t_hi - t_lo) * 3]
                        .rearrange("p (n c) -> p n c", c=3))

            # cross-layer prefetch: the next layer's first ap calls go on
            # the Pool stream BEFORE the gathers (whose h1-write waits would
            # otherwise block them), then gather desc-gen for the first
            # groups so it overlaps this layer's tail.
            if not last_layer:
                emit_gather(l + 1, 0, h1_writes_next, defer_b=True)
                emit_gather(l + 1, 1, h1_writes_next)
            ap_state.pop(l, None)
            return h1_writes_next

        for _rep in range(repeat):
            h1w = h1w0
            pending.clear()
            for l in range(nlayers):
                h1w = _layer(l, h1w, l == nlayers - 1)

    nc.compile()
    return nc


_CACHE = {}


def kernel(**inputs) -> np.ndarray:
    cfg, per_core, post = _prep(inputs)
    key = (cfg["npair"], cfg["S_tot"], cfg["S_dma"], cfg["NB"],
           str(cfg["calls"]), str(cfg["gsub_off"]))
    if key not in _CACHE:
        _CACHE[key] = _build(cfg, per_core[0])
    nc = _CACHE[key]
    res = run_bass_kernel_spmd(nc, per_core, list(range(B)))
    outs = np.empty((B, V, 3), np.float32)
    for m in range(B):
        rows = res.results[m]["out"]
        outs[m] = rows[post["slot_of"][m][np.arange(V)]]
    return outs.reshape(B * V, 3)


if __name__ == "__main__":
    pass


# revision 34
# speedup vs baseline: 1.0409x; 1.0409x over previous
"""Trainium2 Bass kernel for DeformationNetworkGraphConvolutionalFullRes.

Full (unsharded) inputs in, full output out. Data-parallel over the 4 meshes:
core m processes mesh m. Inside each core:

  - vert_align sampling as (S @ F) @ W == S @ (F @ W): per feature map,
    F[C,HW] @ Wslice[C,128] -> G[HW,128] (bf16), then the sparse bilinear
    operator S applied as dense [128px, 512vert] bf16 blocks on the
    TensorEngine (ragged per-chunk schedule), accumulating in PSUM.
    Vertices pre-sorted by image cell.
  - Each GraphConv layer routes its 61440 directed-edge messages through TWO
    independent engines in parallel:
      * DMA half: h1 rows written to HBM in partition-major layout (1KB
        contiguous runs per partition), messages pulled with dma_gather in
        dst-sorted order. Per-tile message counts are padded to the max
        over meshes so the subchunk structure is shared; subchunks are
        packed per scatter GROUP (tile boundaries may fall mid-subchunk),
        each (subchunk, covered-tile) pair scattering with its own
        host-baked dstloc column -- no per-tile ceil padding is gathered.
      * ap half: h1 kept as f32 columns in SBUF (double-banked; 2 source
        windows of 3072 columns each, only sub-window sources are
        ap-routable); gpsimd.ap_gather selects message columns (Pool), PE
        transposes them to row form. Calls are >= the window size so the
        cost is output-bound.
    Both halves are scatter-added per dst tile with one-hot matmuls
    (one is_equal build per group on DVE from a bf16 dstloc table vs an
    iota row; ap one-hots built once, SBUF-resident), accumulating in PSUM
    on top of h0 = W0^T x (+ rank-1 image-encoding term at layer 0); ReLU
    writes the bf16 column-form activations in place. h1 rows/cols for
    layer l+1 are emitted per scatter group of layer l; the double-banked
    h1c removes the end-of-layer flush. Gathers for layer l+1's first
    groups are emitted at the end of layer l so their descriptor
    generation overlaps the boundary. The output projection is fused into
    the last layer's group loop.
"""

import ml_dtypes
import numpy as np
from contextlib import ExitStack

import concourse.bass as bass
import concourse.tile as tile
from concourse import bacc, mybir
from concourse.bass_utils import run_bass_kernel_spmd

# ---------------- problem constants (hardcoded per spec) ----------------
B = 4
V = 10242
E_PER = 30720
HID = 128
MAPS = [(256, 56), (512, 28), (1024, 14), (2048, 7)]  # (C, H==W)
CH_OFF = [0, 256, 768, 1792, 3840]

VP = 10752            # padded vertex count: 84 tiles of 128
NT = VP // 128        # 84 vertex tiles
NVCH = VP // 512      # 21 vertex chunks (sampling)
NW = 2                # ap-gather source windows
W_BASE = [0, 5120]    # window start slots (w0: 10 chunks, w1: 11 chunks)
SUBW = 3072           # ap-routable sub-window (6 chunks of 512)
GT = 8                # dst tiles per scatter group
NGRP = (NT + GT - 1) // GT  # 11 groups (last has 4 tiles)
CALL_SUBS = 32        # max ap-gather call size (subchunks of 128)
AP_FRAC = 0.47        # target fraction of edges through the ap path
PREFETCH = 2          # dma-gather prefetch depth (groups)

F32 = mybir.dt.float32
BF16 = mybir.dt.bfloat16
FP8 = mybir.dt.float8e4
I16 = mybir.dt.int16
AF = mybir.ActivationFunctionType

# sub-window chunks: chunk -> (window, col position in bank)
SUB_CHUNKS = {}
for _w in range(NW):
    for _i in range(SUBW // 512):
        _c = W_BASE[_w] // 512 + _i
        SUB_CHUNKS[_c] = (_w, _w * SUBW + _i * 512)


def _corners(grid, W):
    """grid [V,2] in [-1,1] -> list of (pix_idx int64, weight f32) per corner."""
    x = (grid[:, 0] + 1.0) * 0.5 * (W - 1)
    y = (grid[:, 1] + 1.0) * 0.5 * (W - 1)
    x0f, y0f = np.floor(x), np.floor(y)
    wx1, wy1 = (x - x0f).astype(np.float32), (y - y0f).astype(np.float32)
    wx0, wy0 = 1.0 - wx1, 1.0 - wy1
    x0 = np.clip(x0f, 0, W - 1).astype(np.int64)
    x1 = np.clip(x0f + 1, 0, W - 1).astype(np.int64)
    y0 = np.clip(y0f, 0, W - 1).astype(np.int64)
    y1 = np.clip(y0f + 1, 0, W - 1).astype(np.int64)
    return [
        (y0 * W + x0, wy0 * wx0),
        (y0 * W + x1, wy0 * wx1),
        (y1 * W + x0, wy1 * wx0),
        (y1 * W + x1, wy1 * wx1),
    ]


def _wrap16(idx):
    """int array [n] (n % 16 == 0) -> [128, n/16] wrapped+replicated for the
    8 Q7 cores (idx i at (i%16, i//16))."""
    return np.tile(idx.reshape(-1, 16).T, (8, 1)).astype(np.int16)


def _prep(inputs):
    """Host-side restructuring. Returns (cfg, per_core_aux_list, post)."""
    feats = [inputs["feat1"], inputs["feat2"], inputs["feat3"], inputs["feat4"]]
    av = np.asarray(inputs["aligned_verts"], np.float32)
    verts = np.asarray(inputs["verts_packed"], np.float32)
    enc = np.asarray(inputs["image_enc"], np.float32)
    edges = np.asarray(inputs["edges"], np.int64)

    for bn in ["bottleneck_b", "g0_b0", "g0_b1", "off_b"]:
        assert not np.any(np.asarray(inputs[bn])), f"{bn} nonzero: unsupported"
    assert not np.any(np.asarray(inputs["gb0"])) and not np.any(
        np.asarray(inputs["gb1"])
    ), "gb nonzero: unsupported"

    # per-mesh vertex sort (by finest-map cell): slot = rank in sorted order,
    # pad slots at the very end [V, VP).
    slot_of = []
    corners_all = []
    for m in range(B):
        grid = av[m, :, :2]
        cs = _corners(grid, MAPS[0][1])
        key = cs[0][0]
        sigma = np.argsort(key, kind="stable")
        slot = np.empty(V, np.int64)
        slot[sigma] = np.arange(V)
        slot_of.append(slot)
        corners_all.append([_corners(grid, Wm) for (_, Wm) in MAPS])

    vert_at = []
    for m in range(B):
        va = np.full(VP, -1, np.int64)
        va[slot_of[m][np.arange(V)]] = np.arange(V)
        vert_at.append(va)

    # sampling schedule (ragged: real (map, tile) pairs per chunk, no padding)
    ntile_map = [(Wm * Wm + 127) // 128 for (_, Wm) in MAPS]
    g_off = np.cumsum([0] + ntile_map)
    pairs = []
    for c in range(NVCH):
        lo, hi = c * 512, (c + 1) * 512
        pc = []
        for mi in range(4):
            tiles = set()
            for m in range(B):
                vs = vert_at[m][lo:hi]
                vs = vs[vs >= 0]
                if len(vs):
                    for (pix, _w) in corners_all[m][mi]:
                        tiles.update(np.unique(pix[vs] // 128).tolist())
            for t in sorted(tiles):
                pc.append((mi, t))
        if not pc:
            pc = [(0, 0)]  # zero-weight fallback so PSUM group is well-formed
        pairs.append(pc)
    pair_off = np.concatenate([[0], np.cumsum([len(p) for p in pairs])])
    npair = int(pair_off[-1])
    HNP = max((len(p) + 1) // 2 for p in pairs)

    # graph structure ------------------------------------------------------
    # directed edges in slot space, sorted by (dst tile, src window, src)
    per_mesh_edges = []
    cnt_tw = np.zeros((B, NT, NW), np.int64)      # all msgs per (tile, win)
    cnt_tw_sub = np.zeros((B, NT, NW), np.int64)  # src in sub-window
    for m in range(B):
        e = edges[m * E_PER:(m + 1) * E_PER] - m * V
        a = slot_of[m][e[:, 0]]
        b_ = slot_of[m][e[:, 1]]
        dst = np.concatenate([a, b_])
        src = np.concatenate([b_, a])
        win = (src >= W_BASE[1]).astype(np.int64)
        order = np.lexsort((src, win, dst // 128))
        dst, src, win = dst[order], src[order], win[order]
        per_mesh_edges.append((dst, src, win))
        tl = dst // 128
        insub = (src - np.asarray(W_BASE)[win]) < SUBW
        for t in range(NT):
            sel = tl == t
            for w in range(NW):
                sw = sel & (win == w)
                cnt_tw[m, t, w] = np.sum(sw)
                cnt_tw_sub[m, t, w] = np.sum(sw & insub)

    # ap routing: n_ap[t][w] subchunks through the ap path (shared). Pairs
    # with >= PART_THR leftover sub-window messages get a PARTIAL subchunk
    # (padded with no-op slots) -- cheaper than gathering them over DMA.
    PART_THR = 112
    min_cnt = cnt_tw_sub.min(axis=0)  # [NT, NW]
    n_ap = np.minimum(min_cnt // 128 + (min_cnt % 128 >= PART_THR), 2)
    n_ap = n_ap.astype(np.int64)
    ap_real = np.minimum(128 * n_ap, min_cnt)  # real (non-pad) ap slots
    target_slots = int(AP_FRAC * 2 * E_PER)
    cur = int(ap_real.sum())
    marginal = ap_real - 128 * np.maximum(n_ap - 1, 0)
    order2 = np.argsort(marginal.reshape(-1))
    for idx in order2:
        if cur <= target_slots:
            break
        t, w = divmod(int(idx), NW)
        if n_ap[t, w] > 0:
            n_ap[t, w] -= 1
            nr = int(min(128 * n_ap[t, w], min_cnt[t, w]))
            cur -= int(ap_real[t, w]) - nr
            ap_real[t, w] = nr
    # DMA-half remainder counts, padded to the max over meshes so the
    # subchunk structure (tile spans) is identical on every core. The first
    # SPLIT_GROUPS scatter groups are segmented by source range at SRC_CUT:
    # segment E (src < SRC_CUT) can be gathered before the last h1 writes of
    # the previous layer land; only segment L waits for all of them.
    rcE = np.zeros((B, NT), np.int64)
    rcL = np.zeros((B, NT), np.int64)
    SRC_CUT = (NT - 2 * GT) * 128  # tiles [0, 68): h1 writes 0..16 of 21
    for m in range(B):
        dst, src, win = per_mesh_edges[m]
        pos = 0
        for t in range(NT):
            for w in range(NW):
                c = int(cnt_tw[m, t, w])
                s_ = src[pos:pos + c]
                na = int(ap_real[t, w])
                rem = s_[na:]
                rcE[m, t] += int(np.sum(rem < SRC_CUT))
                rcL[m, t] += int(np.sum(rem >= SRC_CUT))
                pos += c
    rc = rcE + rcL
    assert (rc >= 0).all()
    cnt_pad = rc.max(axis=0)    # [NT] combined (unsegmented groups)
    cnt_padE = rcE.max(axis=0)  # [NT] segmented groups
    cnt_padL = rcL.max(axis=0)
    SPLIT_GROUPS = 1

    def _group_slots(g):
        """Shared per-slot tile array for group g (-1 = pad). Returns
        (tl_slots, cutA) where cutA is the subchunk count of segment E
        (0 for unsegmented groups)."""
        t_lo, t_hi = g * GT, min((g + 1) * GT, NT)
        if g < SPLIT_GROUPS:
            segE = np.concatenate(
                [np.full(int(cnt_padE[t]), t, np.int64)
                 for t in range(t_lo, t_hi)])
            padE = (-len(segE)) % 128
            segL = np.concatenate(
                [np.full(int(cnt_padL[t]), t, np.int64)
                 for t in range(t_lo, t_hi)])
            tl_slots = np.concatenate(
                [segE, np.full(padE, -1, np.int64), segL])
            cutA = (len(segE) + padE) // 128
        else:
            tl_slots = np.concatenate(
                [np.full(int(cnt_pad[t]), t, np.int64)
                 for t in range(t_lo, t_hi)])
            cutA = 0
        padn = (-len(tl_slots)) % 128
        tl_slots = np.concatenate([tl_slots, np.full(padn, -1, np.int64)])
        return tl_slots, cutA

    # group-merged DMA subchunk structure (shared across meshes)
    gsub_off = [0]
    nb_off = [0]
    gmeta = []  # per group: dict(ng, nb, mm=[(j, bloc|None, t_off)], cutA)
    for g in range(NGRP):
        t_lo = g * GT
        tl_slots, cutA = _group_slots(g)
        ng = len(tl_slots) // 128
        mm = []
        nb = 0
        bcols = []
        tk0 = np.full(ng, -1, np.int64)
        for j in range(ng):
            seg = tl_slots[j * 128:(j + 1) * 128]
            tiles_j = sorted(set(int(t) for t in seg if t >= 0))
            if not tiles_j:
                continue
            tk0[j] = tiles_j[0]
            mm.append((j, None, tiles_j[0] - t_lo))
            for t in tiles_j[1:]:
                mm.append((j, nb, t - t_lo))
                bcols.append((j, t))
                nb += 1
        gmeta.append({"ng": ng, "nb": nb, "mm": mm, "bcols": bcols,
                      "tk0": tk0, "cutA": cutA, "tl_slots": tl_slots})
        gsub_off.append(gsub_off[-1] + ng)
        nb_off.append(nb_off[-1] + nb)
    S_dma = gsub_off[-1]
    NB = nb_off[-1]
    MAXSUB_G = max(gm["ng"] for gm in gmeta)
    MAXOH_G = max(gm["ng"] + gm["nb"] for gm in gmeta)

    # ap stream layout per window: subchunk ranges per tile
    ap_off = np.zeros((NT + 1, NW), np.int64)
    for w in range(NW):
        ap_off[1:, w] = np.cumsum(n_ap[:, w])
    S_ap = [int(ap_off[NT, w]) for w in range(NW)]
    S_ap_tot = sum(S_ap)

    # ap gather call partition per stream: whole-GROUP tile ranges, each
    # call <= CALL_SUBS subchunks.
    calls = []  # per w: list of (sub0, sub1, t0, t1, g_start, g_end)
    for w in range(NW):
        cl = []
        g0 = 0
        while g0 < NGRP:
            g1 = g0
            while (g1 < NGRP
                   and ap_off[min((g1 + 1) * GT, NT), w]
                   - ap_off[g0 * GT, w] <= CALL_SUBS):
                g1 += 1
            assert g1 > g0, f"group {g0} stream {w} exceeds CALL_SUBS"
            t0, t1 = g0 * GT, min(g1 * GT, NT)
            s0, s1 = int(ap_off[t0, w]), int(ap_off[t1, w])
            if s1 > s0:
                cl.append((s0, s1, t0, t1, g0, g1 - 1))
            g0 = g1
        calls.append(cl)

    # trmsg rotation safety (shared pool of 3 bufs, merged emission order):
    # tenant i's write must not wait on consumers later than its own readers.
    merged = []
    for w in range(NW):
        for k, c in enumerate(calls[w]):
            merged.append((c[4], c[5], w, k))
    merged.sort()
    for i in range(3, len(merged)):
        assert merged[i][0] > merged[i - 3][1], (
            f"trmsg rotation hazard: {merged[i]} vs {merged[i-3]}")

    S_tot = S_dma + NB + S_ap_tot
    MAXCALL = max(max((c[1] - c[0]) for c in cl) for cl in calls if cl)

    cfg = {"MAXCALL": MAXCALL, "pairs": pairs, "HNP": HNP,
           "pair_off": pair_off.tolist(), "g_off": g_off.tolist(),
           "ntile_map": ntile_map,
           "gsub_off": gsub_off, "gmeta": gmeta, "nb_off": nb_off,
           "S_dma": S_dma, "NB": NB, "MAXSUB_G": MAXSUB_G, "MAXOH_G": MAXOH_G,
           "n_ap": n_ap.tolist(), "ap_off": ap_off.tolist(),
           "S_ap": S_ap, "calls": calls, "S_tot": S_tot, "npair": npair}

    # ---------------- per-core tables ----------------
    per_core = []
    for m in range(B):
        dst, src, win = per_mesh_edges[m]
        ap_idx = [np.zeros(max(16, S_ap[w] * 128), np.int64) for w in range(NW)]
        ap_dl = [np.full(S_ap[w] * 128, -(10 ** 6), np.int64)
                 for w in range(NW)]
        rem_d = [[] for _ in range(NT)]
        rem_s = [[] for _ in range(NT)]
        pos = 0
        for t in range(NT):
            for w in range(NW):
                c = int(cnt_tw[m, t, w])
                d_, s_ = dst[pos:pos + c], src[pos:pos + c]
                na = int(ap_real[t, w])
                o = int(ap_off[t, w]) * 128
                # msgs sorted by src within (t, w): the first na are in the
                # sub-window (na <= cnt_tw_sub[m,t,w] by construction)
                ap_idx[w][o:o + na] = s_[:na] - W_BASE[w]
                ap_dl[w][o:o + na] = d_[:na] - t * 128
                rem_d[t].append(d_[na:])
                rem_s[t].append(s_[na:])
                pos += c
        assert pos == 2 * E_PER
        for w in range(NW):
            assert (ap_idx[w] >= 0).all() and (ap_idx[w] < SUBW).all()

        # group-merged DMA streams (per-tile pad to the shared counts; split
        # groups are segmented by SRC_CUT with segment E padded to a
        # subchunk boundary)
        src_slots = np.zeros(max(16, S_dma * 128), np.int64)
        dl_gr = np.full((S_dma + NB) * 128, -1, np.int64)

        def _tile_seg(t, sel_fn, padto):
            d_ = np.concatenate(rem_d[t])
            s_ = np.concatenate(rem_s[t])
            sel = sel_fn(s_)
            d_, s_ = d_[sel], s_[sel]
            padn = int(padto) - len(d_)
            assert padn >= 0
            return (np.concatenate([d_, np.full(padn, -1, np.int64)]),
                    np.concatenate([s_, np.zeros(padn, np.int64)]))

        for g in range(NGRP):
            t_lo, t_hi = g * GT, min((g + 1) * GT, NT)
            gm = gmeta[g]
            ds, ss = [], []
            if gm["cutA"]:
                for t in range(t_lo, t_hi):
                    d2, s2 = _tile_seg(t, lambda s: s < SRC_CUT, cnt_padE[t])
                    ds.append(d2)
                    ss.append(s2)
                padE = gm["cutA"] * 128 - sum(len(x) for x in ds)
                assert padE >= 0
                ds.append(np.full(padE, -1, np.int64))
                ss.append(np.zeros(padE, np.int64))
                for t in range(t_lo, t_hi):
                    d2, s2 = _tile_seg(t, lambda s: s >= SRC_CUT, cnt_padL[t])
                    ds.append(d2)
                    ss.append(s2)
            else:
                for t in range(t_lo, t_hi):
                    d2, s2 = _tile_seg(t, lambda s: np.ones(len(s), bool),
                                       cnt_pad[t])
                    ds.append(d2)
                    ss.append(s2)
            d_ = np.concatenate(ds) if ds else np.zeros(0, np.int64)
            s_ = np.concatenate(ss) if ss else np.zeros(0, np.int64)
            n = len(d_)
            ng, nb = gm["ng"], gm["nb"]
            so = gsub_off[g] * 128
            src_slots[so:so + n] = s_
            base2 = (gsub_off[g] + nb_off[g]) * 128
            jj = np.arange(n) // 128
            tk0 = np.maximum(np.asarray(gm["tk0"], np.int64), 0)
            dv = np.where(d_ >= 0, d_ - 128 * tk0[jj], -(10 ** 6))
            dl_gr[base2:base2 + n] = dv
            for bi, (j, t) in enumerate(gm["bcols"]):
                col = base2 + (ng + bi) * 128
                lo_s, hi_s = j * 128, min(j * 128 + 128, n)
                seg = d_[lo_s:hi_s]
                dl_gr[col:col + hi_s - lo_s] = np.where(
                    seg >= 0, seg - 128 * t, -(10 ** 6))
        # clamp for bf16 safety: anything outside [0,128) just must not
        # collide with iota values after rounding; keep magnitudes small.
        dl_gr = np.clip(dl_gr, -512, 1024)
        # partition-major h1d row mapping
        rows = (src_slots % 128) * NT + src_slots // 128

        dl_all = np.concatenate([dl_gr] + ap_dl)
        dl_tab = dl_all.reshape(S_tot, 128).T.copy().astype(ml_dtypes.bfloat16)

        srcw = _wrap16(rows)
        apw = [_wrap16(ap_idx[w]) for w in range(NW)]

        # sampling blocks ---------------------------------------------------
        wsc = np.zeros((npair, 128, 512), np.float32)
        pi = 0
        for c in range(NVCH):
            lo = c * 512
            vs_all = vert_at[m][lo:lo + 512]
            jj2 = np.nonzero(vs_all >= 0)[0]
            for (mi, t) in pairs[c]:
                blk = wsc[pi]
                if len(jj2):
                    for (pix, w_) in corners_all[m][mi]:
                        px = pix[vs_all[jj2]]
                        sel = (px >= t * 128) & (px < (t + 1) * 128)
                        j3 = jj2[sel]
                        np.add.at(blk, (pix[vs_all[j3]] - t * 128, j3),
                                  w_[vs_all[j3]])
                pi += 1
        assert pi == npair

        vt = np.zeros((3, VP), np.float32)
        vslots = slot_of[m][np.arange(V)]
        vt[:, vslots] = verts[m * V:(m + 1) * V].T

        bf = ml_dtypes.bfloat16
        aux = {
            "f1": feats[0][m].reshape(256, -1).astype(bf),
            "f2": feats[1][m].reshape(512, -1).astype(bf),
            "f3": feats[2][m].reshape(1024, -1).astype(bf),
            "f4": feats[3][m].reshape(2048, -1).astype(bf),
            "bw": np.asarray(inputs["bottleneck_w"], np.float32).astype(bf),
            "wsc": wsc.reshape(npair * 128, 512).astype(bf),
            "srcw": np.ascontiguousarray(srcw),
            "apw0": np.ascontiguousarray(apw[0]),
            "apw1": np.ascontiguousarray(apw[1]),
            "dstloc": np.ascontiguousarray(dl_tab),
            "iota": np.tile(np.arange(128, dtype=bf), (128, 1)),
            "ident": np.eye(128, dtype=np.float32),
            "vertsT": vt.astype(bf),
            "encc": enc[m].reshape(2, 128).T.copy(),
            "g0w0m": np.asarray(inputs["g0_w0"][:128], np.float32).astype(bf),
            "g0w1m": np.asarray(inputs["g0_w1"][:128], np.float32).astype(bf),
            "g0w0v": np.asarray(inputs["g0_w0"][128:131], np.float32).astype(bf),
            "g0w1v": np.asarray(inputs["g0_w1"][128:131], np.float32).astype(bf),
            "g0w0e": np.ascontiguousarray(
                np.asarray(inputs["g0_w0"][131:387], np.float32)),
            "g0w1e": np.ascontiguousarray(
                np.asarray(inputs["g0_w1"][131:387], np.float32)),
            "gw0": np.ascontiguousarray(
                np.asarray(inputs["gw0"], np.float32).transpose(1, 0, 2)
                .reshape(128, 7 * 128)).astype(bf),
            "gw1": np.ascontiguousarray(
                np.asarray(inputs["gw1"], np.float32).transpose(1, 0, 2)
                .reshape(128, 7 * 128)).astype(bf),
            "offw": np.asarray(inputs["off_w"], np.float32).astype(bf),
        }
        per_core.append(aux)

    post = {"slot_of": slot_of}
    return cfg, per_core, post


def _build(cfg, shapes, nlayers=8, repeat=1):
    """Build the SPMD Bass program (same instruction stream for all cores)."""
    nc = bacc.Bacc("TRN2", target_bir_lowering=False, debug=False, num_devices=B)
    ap = {}
    for name, arr in shapes.items():
        ap[name] = nc.dram_tensor(
            name, list(arr.shape), mybir.dt.from_np(arr.dtype),
            kind="ExternalInput").ap()
    out = nc.dram_tensor("out", [VP, 3], F32, kind="ExternalOutput").ap()
    h1d2 = [nc.dram_tensor("h1da", [VP, HID], BF16).ap(),
            nc.dram_tensor("h1db", [VP, HID], BF16).ap()]

    pairs = cfg["pairs"]
    pair_off = cfg["pair_off"]
    HNP = cfg["HNP"]
    g_off = cfg["g_off"]
    ntile_map = cfg["ntile_map"]
    NGT_ = g_off[4]
    gsub_off = cfg["gsub_off"]
    gmeta = cfg["gmeta"]
    nb_off = cfg["nb_off"]
    S_dma = cfg["S_dma"]
    NB = cfg["NB"]
    MAXSUB_G = cfg["MAXSUB_G"]
    MAXOH_G = cfg["MAXOH_G"]
    n_ap = cfg["n_ap"]
    ap_off = cfg["ap_off"]
    S_ap = cfg["S_ap"]
    calls = cfg["calls"]
    S_tot = cfg["S_tot"]
    S_ap_tot = sum(S_ap)
    MAXCALL = cfg["MAXCALL"]
    ap_base = [S_dma + NB, S_dma + NB + S_ap[0]]

    chunks_by_group = {}
    for c in range(NVCH):
        g = (4 * c + 3) // GT
        chunks_by_group.setdefault(g, []).append(c)

    with tile.TileContext(nc) as tc, ExitStack() as ctx:
        # ---------------- persistent pool ----------------
        pp = ctx.enter_context(tc.tile_pool(name="pers", bufs=1))
        xx = pp.tile([128, VP], BF16, tag="xx")
        h1c0 = pp.tile([128, NW * SUBW], F32, tag="h1c0")
        h1c1 = pp.tile([128, NW * SUBW], F32, tag="h1c1")
        h1c_banks = [h1c0, h1c1]
        oh_ap = pp.tile([128, max(1, S_ap_tot), 128], FP8, tag="ohap")
        srcw_t = pp.tile([128, max(1, S_dma) * 8], I16, tag="srcw")
        apw0_t = pp.tile([128, max(1, S_ap[0]) * 8], I16, tag="apw0")
        apw1_t = pp.tile([128, max(1, S_ap[1]) * 8], I16, tag="apw1")
        apw_t = [apw0_t, apw1_t]
        dstloc_t = pp.tile([128, S_tot, 1], BF16, tag="dstloc")
        iota_t = pp.tile([128, 1, 128], BF16, tag="iota")
        ident_t = pp.tile([128, 128], F32, tag="ident")
        w0_t = pp.tile([128, 7 * 128], BF16, tag="w0")
        w1_t = pp.tile([128, 7 * 128], BF16, tag="w1")
        g0m_t = pp.tile([128, 2 * 128], BF16, tag="g0m")
        g0v_t = pp.tile([3, 256], BF16, tag="g0v")
        offw_t = pp.tile([128, 3], BF16, tag="offw")
        ones_t = pp.tile([1, 512], BF16, tag="ones")
        erow_t = pp.tile([1, 256], BF16, tag="erow")
        encc_t = pp.tile([128, 2], F32, tag="encc")

        nc.sync.dma_start(srcw_t[:], ap["srcw"][:])
        for w in range(NW):
            nc.sync.dma_start(apw_t[w][:], ap[f"apw{w}"][:])
        nc.sync.dma_start(
            dstloc_t[:], ap["dstloc"].rearrange("p (s o) -> p s o", o=1))
        nc.sync.dma_start(iota_t[:].rearrange("p o d -> p (o d)"), ap["iota"][:])
        nc.sync.dma_start(ident_t[:], ap["ident"][:])
        nc.sync.dma_start(w0_t[:], ap["gw0"][:])
        nc.sync.dma_start(w1_t[:], ap["gw1"][:])
        nc.sync.dma_start(g0m_t[:, 0:128], ap["g0w0m"][:])
        nc.sync.dma_start(g0m_t[:, 128:256], ap["g0w1m"][:])
        nc.sync.dma_start(g0v_t[:, 0:128], ap["g0w0v"][:])
        nc.sync.dma_start(g0v_t[:, 128:256], ap["g0w1v"][:])
        nc.sync.dma_start(offw_t[:], ap["offw"][:])
        nc.vector.memset(ones_t[:], 1.0)
        nc.sync.dma_start(encc_t[:], ap["encc"][:])

        # ap one-hots, built once (fp8, resident)
        if S_ap_tot:
            nc.vector.tensor_tensor(
                out=oh_ap[:, :S_ap_tot, :],
                in0=dstloc_t[:, S_dma + NB:S_tot, :]
                .to_broadcast([128, S_ap_tot, 128]),
                in1=iota_t[:].to_broadcast([128, S_ap_tot, 128]),
                op=mybir.AluOpType.is_equal)

        lph = ctx.enter_context(tc.tile_pool(name="hst", bufs=2))
        lpv = ctx.enter_context(tc.tile_pool(name="vv", bufs=1))

        def emit_h1_rows(l, c0, nt4, h1_writes, pool):
            """h1 rows for layer l, tiles [c0, c0+nt4) -> h1d2[l % 2]
            (partition-major: vertex t*128+p lands at row p*NT+t)."""
            h1d = h1d2[l % 2]
            if l == 0:
                vv = lpv.tile([3, 512], BF16, tag="vt")
                nc.sync.dma_start(
                    vv[:, :min(nt4, 4) * 128],
                    ap["vertsT"][:, c0 * 128:(c0 + min(nt4, 4)) * 128])
            hst = lph.tile([128, 1024], BF16, tag="hst")
            for p0 in range(0, nt4, 4):
                pn = min(4, nt4 - p0)
                ph = pool.tile([128, 512], F32, tag="ph")
                for ti in range(pn):
                    t = c0 + p0 + ti
                    sl = slice(ti * 128, (ti + 1) * 128)
                    if l == 0:
                        nc.tensor.matmul(
                            out=ph[:, sl], lhsT=xx[:, t * 128:(t + 1) * 128],
                            rhs=g0m_t[:, 128:256], start=True, stop=False)
                        nc.tensor.matmul(
                            out=ph[:, sl],
                            lhsT=vv[:, (p0 + ti) * 128:(p0 + ti + 1) * 128],
                            rhs=g0v_t[:, 128:256], start=False, stop=False)
                        nc.tensor.matmul(
                            out=ph[:, sl], lhsT=ones_t[:, 0:128],
                            rhs=erow_t[:, 128:256], start=False, stop=True)
                    else:
                        nc.tensor.matmul(
                            out=ph[:, sl], lhsT=xx[:, t * 128:(t + 1) * 128],
                            rhs=w1_t[:, (l - 1) * 128:l * 128],
                            start=True, stop=True)
                nc.scalar.activation(
                    hst[:, p0 * 128:(p0 + pn) * 128], ph[:, :pn * 128],
                    AF.Copy)
            h1_writes.append(nc.sync.dma_start(
                h1d.rearrange("(p n) c -> p n c", p=128)[:, c0:c0 + nt4, :],
                hst[:, :nt4 * 128].rearrange("p (n c) -> p n c", c=128)))

        def emit_h1_cols(l, c, pool):
            """h1 column chunk c for layer l -> its h1c bank (sub-window
            chunks only)."""
            if c not in SUB_CHUNKS:
                return
            _w, pos = SUB_CHUNKS[c]
            bank = h1c_banks[l % 2]
            c0 = c * 512
            cw = 512
            ph = pool.tile([128, 512], F32, tag="ph")
            if l == 0:
                vv = lpv.tile([3, 512], BF16, tag="vt")
                nc.sync.dma_start(vv[:, :cw], ap["vertsT"][:, c0:c0 + cw])
                nc.tensor.matmul(
                    out=ph[:], lhsT=g0m_t[:, 128:256],
                    rhs=xx[:, c0:c0 + cw], start=True, stop=False)
                nc.tensor.matmul(
                    out=ph[:], lhsT=g0v_t[:, 128:256],
                    rhs=vv[:, :cw], start=False, stop=False)
                nc.tensor.matmul(
                    out=ph[:], lhsT=erow_t[:, 128:256],
                    rhs=ones_t[:, :cw], start=False, stop=True)
            else:
                nc.tensor.matmul(
                    out=ph[:], lhsT=w1_t[:, (l - 1) * 128:l * 128],
                    rhs=xx[:, c0:c0 + cw], start=True, stop=True)
            nc.scalar.activation(bank[:, pos:pos + cw], ph[:], AF.Copy)

        samp_done = []
        h1w0 = []
        with ExitStack() as sctx:
            # ---------------- phase 1: sampling ----------------
            sp = sctx.enter_context(tc.tile_pool(name="samp", bufs=1))
            spf = sctx.enter_context(tc.tile_pool(name="sampf", bufs=3))
            spw = sctx.enter_context(tc.tile_pool(name="sampw", bufs=4))
            spp = sctx.enter_context(
                tc.tile_pool(name="sampps", bufs=2, space="PSUM"))
            spp2 = sctx.enter_context(
                tc.tile_pool(name="sampps2", bufs=2, space="PSUM"))

            g0e_t = sp.tile([128, 4 * 128], F32, tag="g0e")
            nc.sync.dma_start(
                g0e_t[:, 0:256].rearrange("p (c h) -> p c h", h=128),
                ap["g0w0e"].rearrange("(c p) h -> p c h", p=128))
            nc.sync.dma_start(
                g0e_t[:, 256:512].rearrange("p (c h) -> p c h", h=128),
                ap["g0w1e"].rearrange("(c p) h -> p c h", p=128))
            for k in range(2):
                pe = spp2.tile([1, 128], F32, tag="pe")
                for cchunk in range(2):
                    nc.tensor.matmul(
                        out=pe[:],
                        lhsT=encc_t[:, cchunk:cchunk + 1],
                        rhs=g0e_t[:, k * 256 + cchunk * 128:
                                  k * 256 + cchunk * 128 + 128],
                        start=(cchunk == 0), stop=(cchunk == 1))
                nc.scalar.activation(erow_t[:, k * 128:(k + 1) * 128], pe[:],
                                     AF.Copy)

            g_sb = sp.tile([128, NGT_ * 128], BF16, tag="gsb")
            for mi, (C, Wm) in enumerate(MAPS):
                HW = Wm * Wm
                ncc = C // 128
                bw_t = spf.tile([128, 16 * 128], BF16, tag="bw")
                nc.sync.dma_start(
                    bw_t[:, :ncc * 128].rearrange("p (c h) -> p c h", h=128),
                    ap["bw"].rearrange("(c p) h -> p c h", p=128)
                    [:, CH_OFF[mi] // 128:CH_OFF[mi] // 128 + ncc, :])
                fm_t = sp.tile([128, 2 * 3136], BF16, tag="fm")
                nc.sync.dma_start(
                    fm_t[:, :ncc * HW].rearrange("p (c hw) -> p c hw", c=ncc),
                    ap[f"f{mi+1}"].rearrange("(c p) hw -> p c hw", p=128))
                for t in range(ntile_map[mi]):
                    p0 = t * 128
                    pcnt = min(128, HW - p0)
                    pg = spp2.tile([128, 128], F32, tag="pg")
                    for cc in range(ncc):
                        nc.tensor.matmul(
                            out=pg[:pcnt, :],
                            lhsT=fm_t[:, cc * HW + p0:cc * HW + p0 + pcnt],
                            rhs=bw_t[:, cc * 128:cc * 128 + 128],
                            start=(cc == 0), stop=(cc == ncc - 1))
                    gt = g_off[mi] + t
                    nc.scalar.activation(
                        g_sb[:pcnt, gt * 128:gt * 128 + 128], pg[:pcnt, :],
                        AF.Copy)

            for c in range(NVCH):
                ps = spp.tile([128, 512], F32, tag="ps")
                pairs_c = pairs[c]
                npc_c = len(pairs_c)
                half = (npc_c + 1) // 2
                wts = []
                for hb in range(2):
                    k0, k1 = hb * half, min((hb + 1) * half, npc_c)
                    wt = spw.tile([128, HNP, 512], BF16, tag="wsc")
                    if k1 > k0:
                        nc.sync.dma_start(
                            wt[:, :k1 - k0, :],
                            ap["wsc"].rearrange("(k p) h -> p k h", p=128)
                            [:, pair_off[c] + k0:pair_off[c] + k1, :])
                    wts.append(wt)
                for k, (mi, t) in enumerate(pairs_c):
                    HW = MAPS[mi][1] ** 2
                    pcnt = min(128, HW - t * 128)
                    gt = g_off[mi] + t
                    nc.tensor.matmul(
                        out=ps[:],
                        lhsT=g_sb[:pcnt, gt * 128:gt * 128 + 128],
                        rhs=wts[k // half][:pcnt, k % half, :],
                        start=(k == 0), stop=(k == npc_c - 1))
                nc.scalar.activation(xx[:, c * 512:(c + 1) * 512], ps[:],
                                     AF.Relu)
                emit_h1_rows(0, c * 4, 4, h1w0, spp)
                emit_h1_cols(0, c, spp)
                samp_done.append(c)

        # ---------------- phase 2: graph conv layers ----------------
        lp = ctx.enter_context(tc.tile_pool(name="msg", bufs=3))
        apb = ctx.enter_context(tc.tile_pool(name="apbuf", bufs=2))
        trp = ctx.enter_context(tc.tile_pool(name="trmsg", bufs=3))
        ohd = ctx.enter_context(tc.tile_pool(name="ohdma", bufs=2))
        psh = ctx.enter_context(tc.tile_pool(name="psh", bufs=2, space="PSUM"))
        pst = ctx.enter_context(tc.tile_pool(name="pst", bufs=2, space="PSUM"))
        psx = ctx.enter_context(tc.tile_pool(name="psx", bufs=2, space="PSUM"))

        pending = {}    # (l, g) -> (msg, ohg) or None
        pending_b = {}  # (l, g) -> (msg, s0, cutA, ng): deferred B segment

        H1W_CUT = (NT - 2 * GT) // 4  # h1 row writes covering tiles < SRC_CUT

        def _emit_gather_part(l, msg, s0, a, b, deps):
            gi = nc.gpsimd.dma_gather(
                out_ap=msg[:, a:b, :],
                in_ap=h1d2[l % 2][:],
                idxs_ap=srcw_t[:, (s0 + a) * 8:(s0 + b) * 8],
                num_idxs=(b - a) * 128,
                num_idxs_reg=(b - a) * 128,
                elem_size=HID,
                single_packet=False,
            )
            for wi in deps:
                tile.add_dep_helper(gi.ins, wi.ins,
                                    reason="h1 RAW: gather after write")

        def emit_gather(l, g, h1_writes, defer_b=False):
            """dma-gather + one-hot build for (layer l, group g). Split
            groups gather segment E (early sources) with a dependency on
            only the first H1W_CUT h1 writes; with defer_b the late-source
            segment is emitted later via emit_deferred_b."""
            s0, s1 = gsub_off[g], gsub_off[g + 1]
            ng = s1 - s0
            if ng == 0:
                pending[(l, g)] = None
                return
            msg = lp.tile([128, MAXSUB_G, 128], BF16, tag="msg")
            cutA = gmeta[g]["cutA"]
            if 0 < cutA < ng:
                _emit_gather_part(l, msg, s0, 0, cutA, h1_writes[:H1W_CUT])
                if defer_b:
                    pending_b[(l, g)] = (msg, s0, cutA, ng)
                else:
                    _emit_gather_part(l, msg, s0, cutA, ng, h1_writes)
            else:
                _emit_gather_part(l, msg, s0, 0, ng, h1_writes)
            noh = ng + gmeta[g]["nb"]
            b2 = gsub_off[g] + nb_off[g]
            ohg = ohd.tile([128, MAXOH_G, 128], FP8, tag="ohg")
            nc.vector.tensor_tensor(
                out=ohg[:, :noh, :],
                in0=dstloc_t[:, b2:b2 + noh, :].to_broadcast([128, noh, 128]),
                in1=iota_t[:].to_broadcast([128, noh, 128]),
                op=mybir.AluOpType.is_equal)
            pending[(l, g)] = (msg, ohg)

        ap_state = {}  # l -> per-layer ap-call emission state

        def _get_ap_state(l):
            if l not in ap_state:
                ap_state[l] = {
                    "next": [0] * NW,
                    "tr": [[None] * len(calls[w]) for w in range(NW)],
                    "flip": [0],
                }
            return ap_state[l]

        def emit_ap_call(l, w, k):
            st = _get_ap_state(l)
            bank = h1c_banks[l % 2]
            s0, s1, _t0, _t1, _gs, _ge = calls[w][k]
            ns = s1 - s0
            buf = apb.tile([128, MAXCALL * 128], F32, tag="apbuf")
            nc.gpsimd.ap_gather(
                out_ap=buf[:, :ns * 128],
                in_ap=bank[:, w * SUBW:(w + 1) * SUBW],
                idxs_ap=apw_t[w][:, s0 * 8:s1 * 8],
                channels=128, num_elems=SUBW, d=1, num_idxs=ns * 128)
            tr = trp.tile([128, MAXCALL, 128], BF16, tag="trmsg")
            st["tr"][w][k] = (tr, s0)
            for j4 in range(0, ns, 4):
                jn = min(4, ns - j4)
                pt = pst.tile([128, 512], F32, tag="pt")
                for j in range(jn):
                    nc.tensor.transpose(
                        pt[:, j * 128:(j + 1) * 128],
                        buf[:, (j4 + j) * 128:(j4 + j + 1) * 128],
                        ident_t[:])
                dst_sl = tr[:, j4:j4 + jn, :].rearrange("p s o -> p (s o)")
                if st["flip"][0] % 2 == 0:
                    nc.vector.tensor_copy(dst_sl, pt[:, :jn * 128])
                else:
                    nc.scalar.activation(dst_sl, pt[:, :jn * 128], AF.Copy)
                st["flip"][0] += 1

        def emit_eligible_calls(l, g):
            # round-robin across windows so group g's trmsg transposes
            # come before deeper-lookahead calls on the in-order engines
            st = _get_ap_state(l)
            while True:
                did = False
                for w in range(NW):
                    if (st["next"][w] < len(calls[w])
                            and calls[w][st["next"][w]][4] <= g + 2):
                        emit_ap_call(l, w, st["next"][w])
                        st["next"][w] += 1
                        did = True
                if not did:
                    break

        def _layer(l, h1_writes, last_layer):
            """Scatter groups for layer l; h1 for layer l+1 is emitted inside
            (pipelined). Returns layer l+1's h1_writes list."""
            h1_writes_next = []
            trmsg_tiles = _get_ap_state(l)["tr"]

            if (l, 0) in pending_b:
                msg_, s0_, cutA_, ng_ = pending_b.pop((l, 0))
                _emit_gather_part(l, msg_, s0_, cutA_, ng_, h1_writes)
            for g in range(min(PREFETCH, NGRP)):
                if (l, g) not in pending:
                    emit_gather(l, g, h1_writes)

            for g in range(NGRP):
                t_lo = g * GT
                t_hi = min((g + 1) * GT, NT)
                emit_eligible_calls(l, g)
                if g + PREFETCH < NGRP:
                    emit_gather(l, g + PREFETCH, h1_writes)

                W_ = (t_hi - t_lo) * 128
                px = psx.tile([128, GT * 128], F32, tag="px")

                got = pending.pop((l, g))
                if got is not None:
                    msg, ohg = got

                mms = []  # entries: (seg_id, kwargs)
                if l == 0:
                    vv2 = lpv.tile([3, GT * 128], BF16, tag="vt2")
                    nc.sync.dma_start(
                        vv2[:, :W_], ap["vertsT"][:, t_lo * 128:t_hi * 128])
                    for seg in range(0, W_, 512):
                        sw = min(512, W_ - seg)
                        c0 = t_lo * 128 + seg
                        mms.append((seg // 512,
                                    dict(out=px[:, seg:seg + sw],
                                         lhsT=g0m_t[:, 0:128],
                                         rhs=xx[:, c0:c0 + sw])))
                        mms.append((seg // 512,
                                    dict(out=px[:, seg:seg + sw],
                                         lhsT=g0v_t[:, 0:128],
                                         rhs=vv2[:, seg:seg + sw])))
                        mms.append((seg // 512,
                                    dict(out=px[:, seg:seg + sw],
                                         lhsT=erow_t[:, 0:128],
                                         rhs=ones_t[:, :sw])))
                else:
                    for seg in range(0, W_, 512):
                        sw = min(512, W_ - seg)
                        c0 = t_lo * 128 + seg
                        mms.append((seg // 512,
                                    dict(out=px[:, seg:seg + sw],
                                         lhsT=w0_t[:, (l - 1) * 128:l * 128],
                                         rhs=xx[:, c0:c0 + sw])))
                # DMA-half scatter (merged subchunks; one oh tile holds the
                # k0 columns [0, ng) and boundary columns [ng, ng+nb))
                if got is not None:
                    ng = gmeta[g]["ng"]
                    for (j, bloc, t_off) in gmeta[g]["mm"]:
                        ohc = j if bloc is None else ng + bloc
                        osl = slice(t_off * 128, (t_off + 1) * 128)
                        mms.append((t_off * 128 // 512,
                                    dict(out=px[:, osl], lhsT=msg[:, j, :],
                                         rhs=ohg[:, ohc, :])))
                # ap-half scatter
                for ti in range(t_hi - t_lo):
                    t = t_lo + ti
                    osl = slice(ti * 128, (ti + 1) * 128)
                    for w in range(NW):
                        na = n_ap[t][w]
                        if na == 0:
                            continue
                        kk = next(
                            i for i, c in enumerate(calls[w])
                            if c[2] <= t < c[3])
                        tr, trs0 = trmsg_tiles[w][kk]
                        for j in range(na):
                            s_loc = ap_off[t][w] - trs0 + j
                            s_ap = ap_off[t][w] + j + (0 if w == 0 else S_ap[0])
                            mms.append((ti * 128 // 512,
                                        dict(out=px[:, osl],
                                             lhsT=tr[:, s_loc, :],
                                             rhs=oh_ap[:, s_ap, :])))
                first_of = {}
                last_of = {}
                for i, (sg, _kw) in enumerate(mms):
                    first_of.setdefault(sg, i)
                    last_of[sg] = i
                for i, (sg, kw) in enumerate(mms):
                    nc.tensor.matmul(start=(first_of[sg] == i),
                                     stop=(last_of[sg] == i),
                                     skip_group_check=True, **kw)
                nc.scalar.activation(xx[:, t_lo * 128:t_hi * 128], px[:, :W_],
                                     AF.Relu)

                # ---- pipelined layer-(l+1) h1 production ----
                if not last_layer:
                    for c0 in range(t_lo, t_hi, 8):
                        emit_h1_rows(l + 1, c0, min(8, t_hi - c0),
                                     h1_writes_next, psh)
                    for c in chunks_by_group.get(g, []):
                        emit_h1_cols(l + 1, c, psh)
                else:
                    ost = lph.tile([128, GT * 3], F32, tag="ost")
                    for ti in range(t_hi - t_lo):
                        t = t_lo + ti
                        po = psh.tile([128, 512], F32, tag="ph")
                        nc.tensor.matmul(out=po[:, :3],
                                         lhsT=xx[:, t * 128:(t + 1) * 128],
                                         rhs=offw_t[:], start=True, stop=True)
                        nc.scalar.activation(ost[:, ti * 3:(ti + 1) * 3],
                                             po[:, :3], AF.Copy)
                    nc.sync.dma_start(
                        out.rearrange("(n p) c -> p n c", p=128)
                        [:, t_lo:t_hi, :],
                        ost[:, :(t_hi - t_lo) * 3]
                        .rearrange("p (n c) -> p n c", c=3))

            # cross-layer prefetch: the next layer's first ap calls go on
            # the Pool stream BEFORE the gathers (whose h1-write waits would
            # otherwise block them), then gather desc-gen for the first
            # groups so it overlaps this layer's tail.
            if not last_layer:
                emit_gather(l + 1, 0, h1_writes_next, defer_b=True)
                emit_gather(l + 1, 1, h1_writes_next)
            ap_state.pop(l, None)
            return h1_writes_next

        for _rep in range(repeat):
            h1w = h1w0
            pending.clear()
            for l in range(nlayers):
                h1w = _layer(l, h1w, l == nlayers - 1)

    nc.compile()
    return nc


_CACHE = {}


def kernel(**inputs) -> np.ndarray:
    cfg, per_core, post = _prep(inputs)
    key = (cfg["npair"], cfg["S_tot"], cfg["S_dma"], cfg["NB"],
           str(cfg["calls"]), str(cfg["gsub_off"]))
    if key not in _CACHE:
        _CACHE[key] = _build(cfg, per_core[0])
    nc = _CACHE[key]
    res = run_bass_kernel_spmd(nc, per_core, list(range(B)))
    outs = np.empty((B, V, 3), np.float32)
    for m in range(B):
        rows = res.results[m]["out"]
        outs[m] = rows[post["slot_of"][m][np.arange(V)]]
    return outs.reshape(B * V, 3)


if __name__ == "__main__":
    pass


# revision 37
# speedup vs baseline: 1.0548x; 1.0134x over previous
"""Trainium2 Bass kernel for DeformationNetworkGraphConvolutionalFullRes.

Full (unsharded) inputs in, full output out. Data-parallel over the 4 meshes:
core m processes mesh m. Inside each core:

  - vert_align sampling as (S @ F) @ W == S @ (F @ W): per feature map,
    F[C,HW] @ Wslice[C,128] -> G[HW,128] (bf16), then the sparse bilinear
    operator S applied as dense [128px, 512vert] bf16 blocks on the
    TensorEngine (ragged per-chunk schedule), accumulating in PSUM.
    Vertices pre-sorted by image cell.
  - Each GraphConv layer routes its 61440 directed-edge messages through TWO
    independent engines in parallel:
      * DMA half: h1 rows written to HBM in partition-major layout (1KB
        contiguous runs per partition), messages pulled with dma_gather in
        dst-sorted order. Per-tile message counts are padded to the max
        over meshes so the subchunk structure is shared; subchunks are
        packed per scatter GROUP (tile boundaries may fall mid-subchunk),
        each (subchunk, covered-tile) pair scattering with its own
        host-baked dstloc column -- no per-tile ceil padding is gathered.
      * ap half: h1 kept as f32 columns in SBUF (double-banked; 2 source
        windows of 3072 columns each, only sub-window sources are
        ap-routable); gpsimd.ap_gather selects message columns (Pool), PE
        transposes them to row form. Calls are >= the window size so the
        cost is output-bound.
    Both halves are scatter-added per dst tile with one-hot matmuls
    (one is_equal build per group on DVE from a bf16 dstloc table vs an
    iota row; ap one-hots built once, SBUF-resident), accumulating in PSUM
    on top of h0 = W0^T x (+ rank-1 image-encoding term at layer 0); ReLU
    writes the bf16 column-form activations in place. h1 rows/cols for
    layer l+1 are emitted per scatter group of layer l; the double-banked
    h1c removes the end-of-layer flush. Gathers for layer l+1's first
    groups are emitted at the end of layer l so their descriptor
    generation overlaps the boundary. The output projection is fused into
    the last layer's group loop.
"""

import ml_dtypes
import numpy as np
from contextlib import ExitStack

import concourse.bass as bass
import concourse.tile as tile
from concourse import bacc, mybir
from concourse.bass_utils import run_bass_kernel_spmd

# ---------------- problem constants (hardcoded per spec) ----------------
B = 4
V = 10242
E_PER = 30720
HID = 128
MAPS = [(256, 56), (512, 28), (1024, 14), (2048, 7)]  # (C, H==W)
CH_OFF = [0, 256, 768, 1792, 3840]

VP = 10752            # padded vertex count: 84 tiles of 128
NT = VP // 128        # 84 vertex tiles
NVCH = VP // 512      # 21 vertex chunks (sampling)
NW = 2                # ap-gather source windows
W_BASE = [0, 5120]    # window start slots (w0: 10 chunks, w1: 11 chunks)
SUBW = 3072           # ap-routable sub-window (6 chunks of 512)
GT = 8                # dst tiles per scatter group
NGRP = (NT + GT - 1) // GT  # 11 groups (last has 4 tiles)
CALL_SUBS = 32        # max ap-gather call size (subchunks of 128)
AP_FRAC = 0.47        # target fraction of edges through the ap path
PREFETCH = 2          # dma-gather prefetch depth (groups)

F32 = mybir.dt.float32
BF16 = mybir.dt.bfloat16
FP8 = mybir.dt.float8e4
I16 = mybir.dt.int16
AF = mybir.ActivationFunctionType

# sub-window chunks: chunk -> (window, col position in bank)
SUB_CHUNKS = {}
for _w in range(NW):
    for _i in range(SUBW // 512):
        _c = W_BASE[_w] // 512 + _i
        SUB_CHUNKS[_c] = (_w, _w * SUBW + _i * 512)


def _corners(grid, W):
    """grid [V,2] in [-1,1] -> list of (pix_idx int64, weight f32) per corner."""
    x = (grid[:, 0] + 1.0) * 0.5 * (W - 1)
    y = (grid[:, 1] + 1.0) * 0.5 * (W - 1)
    x0f, y0f = np.floor(x), np.floor(y)
    wx1, wy1 = (x - x0f).astype(np.float32), (y - y0f).astype(np.float32)
    wx0, wy0 = 1.0 - wx1, 1.0 - wy1
    x0 = np.clip(x0f, 0, W - 1).astype(np.int64)
    x1 = np.clip(x0f + 1, 0, W - 1).astype(np.int64)
    y0 = np.clip(y0f, 0, W - 1).astype(np.int64)
    y1 = np.clip(y0f + 1, 0, W - 1).astype(np.int64)
    return [
        (y0 * W + x0, wy0 * wx0),
        (y0 * W + x1, wy0 * wx1),
        (y1 * W + x0, wy1 * wx0),
        (y1 * W + x1, wy1 * wx1),
    ]


def _wrap16(idx):
    """int array [n] (n % 16 == 0) -> [128, n/16] wrapped+replicated for the
    8 Q7 cores (idx i at (i%16, i//16))."""
    return np.tile(idx.reshape(-1, 16).T, (8, 1)).astype(np.int16)


def _prep(inputs):
    """Host-side restructuring. Returns (cfg, per_core_aux_list, post)."""
    feats = [inputs["feat1"], inputs["feat2"], inputs["feat3"], inputs["feat4"]]
    av = np.asarray(inputs["aligned_verts"], np.float32)
    verts = np.asarray(inputs["verts_packed"], np.float32)
    enc = np.asarray(inputs["image_enc"], np.float32)
    edges = np.asarray(inputs["edges"], np.int64)

    for bn in ["bottleneck_b", "g0_b0", "g0_b1", "off_b"]:
        assert not np.any(np.asarray(inputs[bn])), f"{bn} nonzero: unsupported"
    assert not np.any(np.asarray(inputs["gb0"])) and not np.any(
        np.asarray(inputs["gb1"])
    ), "gb nonzero: unsupported"

    # per-mesh vertex sort (by finest-map cell): slot = rank in sorted order,
    # pad slots at the very end [V, VP).
    slot_of = []
    corners_all = []
    for m in range(B):
        grid = av[m, :, :2]
        cs = _corners(grid, MAPS[0][1])
        key = cs[0][0]
        sigma = np.argsort(key, kind="stable")
        slot = np.empty(V, np.int64)
        slot[sigma] = np.arange(V)
        slot_of.append(slot)
        corners_all.append([_corners(grid, Wm) for (_, Wm) in MAPS])

    vert_at = []
    for m in range(B):
        va = np.full(VP, -1, np.int64)
        va[slot_of[m][np.arange(V)]] = np.arange(V)
        vert_at.append(va)

    # sampling schedule (ragged: real (map, tile) pairs per chunk, no padding)
    ntile_map = [(Wm * Wm + 127) // 128 for (_, Wm) in MAPS]
    g_off = np.cumsum([0] + ntile_map)
    pairs = []
    for c in range(NVCH):
        lo, hi = c * 512, (c + 1) * 512
        pc = []
        for mi in range(4):
            tiles = set()
            for m in range(B):
                vs = vert_at[m][lo:hi]
                vs = vs[vs >= 0]
                if len(vs):
                    for (pix, _w) in corners_all[m][mi]:
                        tiles.update(np.unique(pix[vs] // 128).tolist())
            for t in sorted(tiles):
                pc.append((mi, t))
        if not pc:
            pc = [(0, 0)]  # zero-weight fallback so PSUM group is well-formed
        pairs.append(pc)
    pair_off = np.concatenate([[0], np.cumsum([len(p) for p in pairs])])
    npair = int(pair_off[-1])
    HNP = max((len(p) + 1) // 2 for p in pairs)

    # graph structure ------------------------------------------------------
    # directed edges in slot space, sorted by (dst tile, src window, src)
    per_mesh_edges = []
    cnt_tw = np.zeros((B, NT, NW), np.int64)      # all msgs per (tile, win)
    cnt_tw_sub = np.zeros((B, NT, NW), np.int64)  # src in sub-window
    for m in range(B):
        e = edges[m * E_PER:(m + 1) * E_PER] - m * V
        a = slot_of[m][e[:, 0]]
        b_ = slot_of[m][e[:, 1]]
        dst = np.concatenate([a, b_])
        src = np.concatenate([b_, a])
        win = (src >= W_BASE[1]).astype(np.int64)
        order = np.lexsort((src, win, dst // 128))
        dst, src, win = dst[order], src[order], win[order]
        per_mesh_edges.append((dst, src, win))
        tl = dst // 128
        insub = (src - np.asarray(W_BASE)[win]) < SUBW
        for t in range(NT):
            sel = tl == t
            for w in range(NW):
                sw = sel & (win == w)
                cnt_tw[m, t, w] = np.sum(sw)
                cnt_tw_sub[m, t, w] = np.sum(sw & insub)

    # ap routing: n_ap[t][w] subchunks through the ap path (shared). Pairs
    # with >= PART_THR leftover sub-window messages get a PARTIAL subchunk
    # (padded with no-op slots) -- cheaper than gathering them over DMA.
    PART_THR = 112
    min_cnt = cnt_tw_sub.min(axis=0)  # [NT, NW]
    n_ap = np.minimum(min_cnt // 128 + (min_cnt % 128 >= PART_THR), 2)
    n_ap = n_ap.astype(np.int64)
    ap_real = np.minimum(128 * n_ap, min_cnt)  # real (non-pad) ap slots
    target_slots = int(AP_FRAC * 2 * E_PER)
    cur = int(ap_real.sum())
    marginal = ap_real - 128 * np.maximum(n_ap - 1, 0)
    order2 = np.argsort(marginal.reshape(-1))
    for idx in order2:
        if cur <= target_slots:
            break
        t, w = divmod(int(idx), NW)
        if n_ap[t, w] > 0:
            n_ap[t, w] -= 1
            nr = int(min(128 * n_ap[t, w], min_cnt[t, w]))
            cur -= int(ap_real[t, w]) - nr
            ap_real[t, w] = nr
    # DMA-half remainder counts, padded to the max over meshes so the
    # subchunk structure (tile spans) is identical on every core. The first
    # SPLIT_GROUPS scatter groups are segmented by source range at SRC_CUT:
    # segment E (src < SRC_CUT) can be gathered before the last h1 writes of
    # the previous layer land; only segment L waits for all of them.
    rcE = np.zeros((B, NT), np.int64)
    rcL = np.zeros((B, NT), np.int64)
    SRC_CUT = (NT - 2 * GT) * 128  # tiles [0, 68): h1 writes 0..16 of 21
    for m in range(B):
        dst, src, win = per_mesh_edges[m]
        pos = 0
        for t in range(NT):
            for w in range(NW):
                c = int(cnt_tw[m, t, w])
                s_ = src[pos:pos + c]
                na = int(ap_real[t, w])
                rem = s_[na:]
                rcE[m, t] += int(np.sum(rem < SRC_CUT))
                rcL[m, t] += int(np.sum(rem >= SRC_CUT))
                pos += c
    rc = rcE + rcL
    assert (rc >= 0).all()
    cnt_pad = rc.max(axis=0)    # [NT] combined (unsegmented groups)
    cnt_padE = rcE.max(axis=0)  # [NT] segmented groups
    cnt_padL = rcL.max(axis=0)
    SPLIT_GROUPS = 1

    def _group_slots(g):
        """Shared per-slot tile array for group g (-1 = pad). Returns
        (tl_slots, cutA) where cutA is the subchunk count of segment E
        (0 for unsegmented groups)."""
        t_lo, t_hi = g * GT, min((g + 1) * GT, NT)
        if g < SPLIT_GROUPS:
            segE = np.concatenate(
                [np.full(int(cnt_padE[t]), t, np.int64)
                 for t in range(t_lo, t_hi)])
            padE = (-len(segE)) % 128
            segL = np.concatenate(
                [np.full(int(cnt_padL[t]), t, np.int64)
                 for t in range(t_lo, t_hi)])
            tl_slots = np.concatenate(
                [segE, np.full(padE, -1, np.int64), segL])
            cutA = (len(segE) + padE) // 128
        else:
            tl_slots = np.concatenate(
                [np.full(int(cnt_pad[t]), t, np.int64)
                 for t in range(t_lo, t_hi)])
            cutA = 0
        padn = (-len(tl_slots)) % 128
        tl_slots = np.concatenate([tl_slots, np.full(padn, -1, np.int64)])
        return tl_slots, cutA

    # group-merged DMA subchunk structure (shared across meshes)
    gsub_off = [0]
    nb_off = [0]
    gmeta = []  # per group: dict(ng, nb, mm=[(j, bloc|None, t_off)], cutA)
    for g in range(NGRP):
        t_lo = g * GT
        tl_slots, cutA = _group_slots(g)
        ng = len(tl_slots) // 128
        mm = []
        nb = 0
        bcols = []
        tk0 = np.full(ng, -1, np.int64)
        for j in range(ng):
            seg = tl_slots[j * 128:(j + 1) * 128]
            tiles_j = sorted(set(int(t) for t in seg if t >= 0))
            if not tiles_j:
                continue
            tk0[j] = tiles_j[0]
            mm.append((j, None, tiles_j[0] - t_lo))
            for t in tiles_j[1:]:
                mm.append((j, nb, t - t_lo))
                bcols.append((j, t))
                nb += 1
        gmeta.append({"ng": ng, "nb": nb, "mm": mm, "bcols": bcols,
                      "tk0": tk0, "cutA": cutA, "tl_slots": tl_slots})
        gsub_off.append(gsub_off[-1] + ng)
        nb_off.append(nb_off[-1] + nb)
    S_dma = gsub_off[-1]
    NB = nb_off[-1]
    MAXSUB_G = max(gm["ng"] for gm in gmeta)
    MAXOH_G = max(gm["ng"] + gm["nb"] for gm in gmeta)

    # ap stream layout per window: subchunk ranges per tile
    ap_off = np.zeros((NT + 1, NW), np.int64)
    for w in range(NW):
        ap_off[1:, w] = np.cumsum(n_ap[:, w])
    S_ap = [int(ap_off[NT, w]) for w in range(NW)]
    S_ap_tot = sum(S_ap)

    # ap gather call partition per stream: whole-GROUP tile ranges, each
    # call <= CALL_SUBS subchunks.
    calls = []  # per w: list of (sub0, sub1, t0, t1, g_start, g_end)
    for w in range(NW):
        cl = []
        g0 = 0
        while g0 < NGRP:
            g1 = g0
            while (g1 < NGRP
                   and ap_off[min((g1 + 1) * GT, NT), w]
                   - ap_off[g0 * GT, w] <= CALL_SUBS):
                g1 += 1
            assert g1 > g0, f"group {g0} stream {w} exceeds CALL_SUBS"
            t0, t1 = g0 * GT, min(g1 * GT, NT)
            s0, s1 = int(ap_off[t0, w]), int(ap_off[t1, w])
            if s1 > s0:
                cl.append((s0, s1, t0, t1, g0, g1 - 1))
            g0 = g1
        calls.append(cl)

    # trmsg rotation safety (shared pool of 3 bufs, merged emission order):
    # tenant i's write must not wait on consumers later than its own readers.
    merged = []
    for w in range(NW):
        for k, c in enumerate(calls[w]):
            merged.append((c[4], c[5], w, k))
    merged.sort()
    for i in range(3, len(merged)):
        assert merged[i][0] > merged[i - 3][1], (
            f"trmsg rotation hazard: {merged[i]} vs {merged[i-3]}")

    S_tot = S_dma + NB + S_ap_tot
    MAXCALL = max(max((c[1] - c[0]) for c in cl) for cl in calls if cl)

    cfg = {"MAXCALL": MAXCALL, "pairs": pairs, "HNP": HNP,
           "pair_off": pair_off.tolist(), "g_off": g_off.tolist(),
           "ntile_map": ntile_map,
           "gsub_off": gsub_off, "gmeta": gmeta, "nb_off": nb_off,
           "S_dma": S_dma, "NB": NB, "MAXSUB_G": MAXSUB_G, "MAXOH_G": MAXOH_G,
           "n_ap": n_ap.tolist(), "ap_off": ap_off.tolist(),
           "S_ap": S_ap, "calls": calls, "S_tot": S_tot, "npair": npair}

    # ---------------- per-core tables ----------------
    per_core = []
    for m in range(B):
        dst, src, win = per_mesh_edges[m]
        ap_idx = [np.zeros(max(16, S_ap[w] * 128), np.int64) for w in range(NW)]
        ap_dl = [np.full(S_ap[w] * 128, -(10 ** 6), np.int64)
                 for w in range(NW)]
        rem_d = [[] for _ in range(NT)]
        rem_s = [[] for _ in range(NT)]
        pos = 0
        for t in range(NT):
            for w in range(NW):
                c = int(cnt_tw[m, t, w])
                d_, s_ = dst[pos:pos + c], src[pos:pos + c]
                na = int(ap_real[t, w])
                o = int(ap_off[t, w]) * 128
                # msgs sorted by src within (t, w): the first na are in the
                # sub-window (na <= cnt_tw_sub[m,t,w] by construction)
                ap_idx[w][o:o + na] = s_[:na] - W_BASE[w]
                ap_dl[w][o:o + na] = d_[:na] - t * 128
                rem_d[t].append(d_[na:])
                rem_s[t].append(s_[na:])
                pos += c
        assert pos == 2 * E_PER
        for w in range(NW):
            assert (ap_idx[w] >= 0).all() and (ap_idx[w] < SUBW).all()

        # group-merged DMA streams (per-tile pad to the shared counts; split
        # groups are segmented by SRC_CUT with segment E padded to a
        # subchunk boundary)
        src_slots = np.zeros(max(16, S_dma * 128), np.int64)
        dl_gr = np.full((S_dma + NB) * 128, -1, np.int64)

        def _tile_seg(t, sel_fn, padto):
            d_ = np.concatenate(rem_d[t])
            s_ = np.concatenate(rem_s[t])
            sel = sel_fn(s_)
            d_, s_ = d_[sel], s_[sel]
            padn = int(padto) - len(d_)
            assert padn >= 0
            return (np.concatenate([d_, np.full(padn, -1, np.int64)]),
                    np.concatenate([s_, np.zeros(padn, np.int64)]))

        for g in range(NGRP):
            t_lo, t_hi = g * GT, min((g + 1) * GT, NT)
            gm = gmeta[g]
            ds, ss = [], []
            if gm["cutA"]:
                for t in range(t_lo, t_hi):
                    d2, s2 = _tile_seg(t, lambda s: s < SRC_CUT, cnt_padE[t])
                    ds.append(d2)
                    ss.append(s2)
                padE = gm["cutA"] * 128 - sum(len(x) for x in ds)
                assert padE >= 0
                ds.append(np.full(padE, -1, np.int64))
                ss.append(np.zeros(padE, np.int64))
                for t in range(t_lo, t_hi):
                    d2, s2 = _tile_seg(t, lambda s: s >= SRC_CUT, cnt_padL[t])
                    ds.append(d2)
                    ss.append(s2)
            else:
                for t in range(t_lo, t_hi):
                    d2, s2 = _tile_seg(t, lambda s: np.ones(len(s), bool),
                                       cnt_pad[t])
                    ds.append(d2)
                    ss.append(s2)
            d_ = np.concatenate(ds) if ds else np.zeros(0, np.int64)
            s_ = np.concatenate(ss) if ss else np.zeros(0, np.int64)
            n = len(d_)
            ng, nb = gm["ng"], gm["nb"]
            so = gsub_off[g] * 128
            src_slots[so:so + n] = s_
            base2 = (gsub_off[g] + nb_off[g]) * 128
            jj = np.arange(n) // 128
            tk0 = np.maximum(np.asarray(gm["tk0"], np.int64), 0)
            dv = np.where(d_ >= 0, d_ - 128 * tk0[jj], -(10 ** 6))
            dl_gr[base2:base2 + n] = dv
            for bi, (j, t) in enumerate(gm["bcols"]):
                col = base2 + (ng + bi) * 128
                lo_s, hi_s = j * 128, min(j * 128 + 128, n)
                seg = d_[lo_s:hi_s]
                dl_gr[col:col + hi_s - lo_s] = np.where(
                    seg >= 0, seg - 128 * t, -(10 ** 6))
        # clamp for bf16 safety: anything outside [0,128) just must not
        # collide with iota values after rounding; keep magnitudes small.
        dl_gr = np.clip(dl_gr, -512, 1024)
        # partition-major h1d row mapping
        rows = (src_slots % 128) * NT + src_slots // 128

        dl_all = np.concatenate([dl_gr] + ap_dl)
        dl_tab = dl_all.reshape(S_tot, 128).T.copy().astype(ml_dtypes.bfloat16)

        srcw = _wrap16(rows)
        apw = [_wrap16(ap_idx[w]) for w in range(NW)]

        # sampling blocks ---------------------------------------------------
        wsc = np.zeros((npair, 128, 512), np.float32)
        pi = 0
        for c in range(NVCH):
            lo = c * 512
            vs_all = vert_at[m][lo:lo + 512]
            jj2 = np.nonzero(vs_all >= 0)[0]
            for (mi, t) in pairs[c]:
                blk = wsc[pi]
                if len(jj2):
                    for (pix, w_) in corners_all[m][mi]:
                        px = pix[vs_all[jj2]]
                        sel = (px >= t * 128) & (px < (t + 1) * 128)
                        j3 = jj2[sel]
                        np.add.at(blk, (pix[vs_all[j3]] - t * 128, j3),
                                  w_[vs_all[j3]])
                pi += 1
        assert pi == npair

        vt = np.zeros((3, VP), np.float32)
        vslots = slot_of[m][np.arange(V)]
        vt[:, vslots] = verts[m * V:(m + 1) * V].T

        bf = ml_dtypes.bfloat16
        aux = {
            "f1": feats[0][m].reshape(256, -1).astype(bf),
            "f2": feats[1][m].reshape(512, -1).astype(bf),
            "f3": feats[2][m].reshape(1024, -1).astype(bf),
            "f4": feats[3][m].reshape(2048, -1).astype(bf),
            "bw": np.asarray(inputs["bottleneck_w"], np.float32).astype(bf),
            "wsc": wsc.reshape(npair * 128, 512).astype(bf),
            "srcw": np.ascontiguousarray(srcw),
            "apw0": np.ascontiguousarray(apw[0]),
            "apw1": np.ascontiguousarray(apw[1]),
            "dstloc": np.ascontiguousarray(dl_tab),
            "iota": np.tile(np.arange(128, dtype=bf), (128, 1)),
            "ident": np.eye(128, dtype=np.float32),
            "vertsT": vt.astype(bf),
            "encc": enc[m].reshape(2, 128).T.copy(),
            "g0w0m": np.asarray(inputs["g0_w0"][:128], np.float32).astype(bf),
            "g0w1m": np.asarray(inputs["g0_w1"][:128], np.float32).astype(bf),
            "g0w0v": np.asarray(inputs["g0_w0"][128:131], np.float32).astype(bf),
            "g0w1v": np.asarray(inputs["g0_w1"][128:131], np.float32).astype(bf),
            "g0w0e": np.ascontiguousarray(
                np.asarray(inputs["g0_w0"][131:387], np.float32)),
            "g0w1e": np.ascontiguousarray(
                np.asarray(inputs["g0_w1"][131:387], np.float32)),
            "gw0": np.ascontiguousarray(
                np.asarray(inputs["gw0"], np.float32).transpose(1, 0, 2)
                .reshape(128, 7 * 128)).astype(bf),
            "gw1": np.ascontiguousarray(
                np.asarray(inputs["gw1"], np.float32).transpose(1, 0, 2)
                .reshape(128, 7 * 128)).astype(bf),
            "offw": np.asarray(inputs["off_w"], np.float32).astype(bf),
        }
        per_core.append(aux)

    post = {"slot_of": slot_of}
    return cfg, per_core, post


def _build(cfg, shapes, nlayers=8, repeat=1):
    """Build the SPMD Bass program (same instruction stream for all cores)."""
    nc = bacc.Bacc("TRN2", target_bir_lowering=False, debug=False, num_devices=B)
    ap = {}
    for name, arr in shapes.items():
        ap[name] = nc.dram_tensor(
            name, list(arr.shape), mybir.dt.from_np(arr.dtype),
            kind="ExternalInput").ap()
    out = nc.dram_tensor("out", [VP, 3], F32, kind="ExternalOutput").ap()
    h1d2 = [nc.dram_tensor("h1da", [VP, HID], BF16).ap(),
            nc.dram_tensor("h1db", [VP, HID], BF16).ap()]

    pairs = cfg["pairs"]
    pair_off = cfg["pair_off"]
    HNP = cfg["HNP"]
    g_off = cfg["g_off"]
    ntile_map = cfg["ntile_map"]
    NGT_ = g_off[4]
    gsub_off = cfg["gsub_off"]
    gmeta = cfg["gmeta"]
    nb_off = cfg["nb_off"]
    S_dma = cfg["S_dma"]
    NB = cfg["NB"]
    MAXSUB_G = cfg["MAXSUB_G"]
    MAXOH_G = cfg["MAXOH_G"]
    n_ap = cfg["n_ap"]
    ap_off = cfg["ap_off"]
    S_ap = cfg["S_ap"]
    calls = cfg["calls"]
    S_tot = cfg["S_tot"]
    S_ap_tot = sum(S_ap)
    MAXCALL = cfg["MAXCALL"]
    ap_base = [S_dma + NB, S_dma + NB + S_ap[0]]

    chunks_by_group = {}
    for c in range(NVCH):
        g = (4 * c + 3) // GT
        chunks_by_group.setdefault(g, []).append(c)

    with tile.TileContext(nc) as tc, ExitStack() as ctx:
        # ---------------- persistent pool ----------------
        pp = ctx.enter_context(tc.tile_pool(name="pers", bufs=1))
        xx = pp.tile([128, VP], BF16, tag="xx")
        h1c0 = pp.tile([128, NW * SUBW], F32, tag="h1c0")
        h1c1 = pp.tile([128, NW * SUBW], F32, tag="h1c1")
        h1c_banks = [h1c0, h1c1]
        oh_ap = pp.tile([128, max(1, S_ap_tot), 128], FP8, tag="ohap")
        srcw_t = pp.tile([128, max(1, S_dma) * 8], I16, tag="srcw")
        apw0_t = pp.tile([128, max(1, S_ap[0]) * 8], I16, tag="apw0")
        apw1_t = pp.tile([128, max(1, S_ap[1]) * 8], I16, tag="apw1")
        apw_t = [apw0_t, apw1_t]
        dstloc_t = pp.tile([128, S_tot, 1], BF16, tag="dstloc")
        iota_t = pp.tile([128, 1, 128], BF16, tag="iota")
        ident_t = pp.tile([128, 128], F32, tag="ident")
        w0_t = pp.tile([128, 7 * 128], BF16, tag="w0")
        w1_t = pp.tile([128, 7 * 128], BF16, tag="w1")
        g0m_t = pp.tile([128, 2 * 128], BF16, tag="g0m")
        g0v_t = pp.tile([3, 256], BF16, tag="g0v")
        offw_t = pp.tile([128, 3], BF16, tag="offw")
        ones_t = pp.tile([1, 512], BF16, tag="ones")
        erow_t = pp.tile([1, 256], BF16, tag="erow")
        encc_t = pp.tile([128, 2], F32, tag="encc")

        nc.sync.dma_start(srcw_t[:], ap["srcw"][:])
        for w in range(NW):
            nc.sync.dma_start(apw_t[w][:], ap[f"apw{w}"][:])
        nc.sync.dma_start(
            dstloc_t[:], ap["dstloc"].rearrange("p (s o) -> p s o", o=1))
        nc.sync.dma_start(iota_t[:].rearrange("p o d -> p (o d)"), ap["iota"][:])
        nc.sync.dma_start(ident_t[:], ap["ident"][:])
        nc.sync.dma_start(w0_t[:], ap["gw0"][:])
        nc.sync.dma_start(w1_t[:], ap["gw1"][:])
        nc.sync.dma_start(g0m_t[:, 0:128], ap["g0w0m"][:])
        nc.sync.dma_start(g0m_t[:, 128:256], ap["g0w1m"][:])
        nc.sync.dma_start(g0v_t[:, 0:128], ap["g0w0v"][:])
        nc.sync.dma_start(g0v_t[:, 128:256], ap["g0w1v"][:])
        nc.sync.dma_start(offw_t[:], ap["offw"][:])
        nc.vector.memset(ones_t[:], 1.0)
        nc.sync.dma_start(encc_t[:], ap["encc"][:])

        # ap one-hots, built once (fp8, resident)
        if S_ap_tot:
            nc.vector.tensor_tensor(
                out=oh_ap[:, :S_ap_tot, :],
                in0=dstloc_t[:, S_dma + NB:S_tot, :]
                .to_broadcast([128, S_ap_tot, 128]),
                in1=iota_t[:].to_broadcast([128, S_ap_tot, 128]),
                op=mybir.AluOpType.is_equal)

        lph = ctx.enter_context(tc.tile_pool(name="hst", bufs=2))
        lpv = ctx.enter_context(tc.tile_pool(name="vv", bufs=1))

        def emit_h1_rows(l, c0, nt4, h1_writes, pool):
            """h1 rows for layer l, tiles [c0, c0+nt4) -> h1d2[l % 2]
            (partition-major: vertex t*128+p lands at row p*NT+t)."""
            h1d = h1d2[l % 2]
            if l == 0:
                vv = lpv.tile([3, 512], BF16, tag="vt")
                nc.sync.dma_start(
                    vv[:, :min(nt4, 4) * 128],
                    ap["vertsT"][:, c0 * 128:(c0 + min(nt4, 4)) * 128])
            hst = lph.tile([128, 1024], BF16, tag="hst")
            for p0 in range(0, nt4, 4):
                pn = min(4, nt4 - p0)
                ph = pool.tile([128, 512], F32, tag="ph")
                for ti in range(pn):
                    t = c0 + p0 + ti
                    sl = slice(ti * 128, (ti + 1) * 128)
                    if l == 0:
                        nc.tensor.matmul(
                            out=ph[:, sl], lhsT=xx[:, t * 128:(t + 1) * 128],
                            rhs=g0m_t[:, 128:256], start=True, stop=False)
                        nc.tensor.matmul(
                            out=ph[:, sl],
                            lhsT=vv[:, (p0 + ti) * 128:(p0 + ti + 1) * 128],
                            rhs=g0v_t[:, 128:256], start=False, stop=False)
                        nc.tensor.matmul(
                            out=ph[:, sl], lhsT=ones_t[:, 0:128],
                            rhs=erow_t[:, 128:256], start=False, stop=True)
                    else:
                        nc.tensor.matmul(
                            out=ph[:, sl], lhsT=xx[:, t * 128:(t + 1) * 128],
                            rhs=w1_t[:, (l - 1) * 128:l * 128],
                            start=True, stop=True)
                nc.scalar.activation(
                    hst[:, p0 * 128:(p0 + pn) * 128], ph[:, :pn * 128],
                    AF.Copy)
            h1_writes.append(nc.sync.dma_start(
                h1d.rearrange("(p n) c -> p n c", p=128)[:, c0:c0 + nt4, :],
                hst[:, :nt4 * 128].rearrange("p (n c) -> p n c", c=128)))

        def emit_h1_cols(l, c, pool):
            """h1 column chunk c for layer l -> its h1c bank (sub-window
            chunks only)."""
            if c not in SUB_CHUNKS:
                return
            _w, pos = SUB_CHUNKS[c]
            bank = h1c_banks[l % 2]
            c0 = c * 512
            cw = 512
            ph = pool.tile([128, 512], F32, tag="ph")
            if l == 0:
                vv = lpv.tile([3, 512], BF16, tag="vt")
                nc.sync.dma_start(vv[:, :cw], ap["vertsT"][:, c0:c0 + cw])
                nc.tensor.matmul(
                    out=ph[:], lhsT=g0m_t[:, 128:256],
                    rhs=xx[:, c0:c0 + cw], start=True, stop=False)
                nc.tensor.matmul(
                    out=ph[:], lhsT=g0v_t[:, 128:256],
                    rhs=vv[:, :cw], start=False, stop=False)
                nc.tensor.matmul(
                    out=ph[:], lhsT=erow_t[:, 128:256],
                    rhs=ones_t[:, :cw], start=False, stop=True)
            else:
                nc.tensor.matmul(
                    out=ph[:], lhsT=w1_t[:, (l - 1) * 128:l * 128],
                    rhs=xx[:, c0:c0 + cw], start=True, stop=True)
            nc.scalar.activation(bank[:, pos:pos + cw], ph[:], AF.Copy)

        samp_done = []
        h1w0 = []
        with ExitStack() as sctx:
            # ---------------- phase 1: sampling ----------------
            sp = sctx.enter_context(tc.tile_pool(name="samp", bufs=1))
            spf = sctx.enter_context(tc.tile_pool(name="sampf", bufs=3))
            spw = sctx.enter_context(tc.tile_pool(name="sampw", bufs=4))
            spm = sctx.enter_context(tc.tile_pool(name="sampfm", bufs=2))
            spp = sctx.enter_context(
                tc.tile_pool(name="sampps", bufs=2, space="PSUM"))
            spp2 = sctx.enter_context(
                tc.tile_pool(name="sampps2", bufs=2, space="PSUM"))

            g0e_t = sp.tile([128, 4 * 128], F32, tag="g0e")
            nc.sync.dma_start(
                g0e_t[:, 0:256].rearrange("p (c h) -> p c h", h=128),
                ap["g0w0e"].rearrange("(c p) h -> p c h", p=128))
            nc.sync.dma_start(
                g0e_t[:, 256:512].rearrange("p (c h) -> p c h", h=128),
                ap["g0w1e"].rearrange("(c p) h -> p c h", p=128))
            for k in range(2):
                pe = spp2.tile([1, 128], F32, tag="pe")
                for cchunk in range(2):
                    nc.tensor.matmul(
                        out=pe[:],
                        lhsT=encc_t[:, cchunk:cchunk + 1],
                        rhs=g0e_t[:, k * 256 + cchunk * 128:
                                  k * 256 + cchunk * 128 + 128],
                        start=(cchunk == 0), stop=(cchunk == 1))
                nc.scalar.activation(erow_t[:, k * 128:(k + 1) * 128], pe[:],
                                     AF.Copy)

            g_sb = sp.tile([128, NGT_ * 128], BF16, tag="gsb")
            for mi, (C, Wm) in enumerate(MAPS):
                HW = Wm * Wm
                ncc = C // 128
                bw_t = spf.tile([128, 16 * 128], BF16, tag="bw")
                nc.sync.dma_start(
                    bw_t[:, :ncc * 128].rearrange("p (c h) -> p c h", h=128),
                    ap["bw"].rearrange("(c p) h -> p c h", p=128)
                    [:, CH_OFF[mi] // 128:CH_OFF[mi] // 128 + ncc, :])
                fm_t = spm.tile([128, 2 * 3136], BF16, tag="fm")
                nc.sync.dma_start(
                    fm_t[:, :ncc * HW].rearrange("p (c hw) -> p c hw", c=ncc),
                    ap[f"f{mi+1}"].rearrange("(c p) hw -> p c hw", p=128))
                for t in range(ntile_map[mi]):
                    p0 = t * 128
                    pcnt = min(128, HW - p0)
                    pg = spp2.tile([128, 128], F32, tag="pg")
                    for cc in range(ncc):
                        nc.tensor.matmul(
                            out=pg[:pcnt, :],
                            lhsT=fm_t[:, cc * HW + p0:cc * HW + p0 + pcnt],
                            rhs=bw_t[:, cc * 128:cc * 128 + 128],
                            start=(cc == 0), stop=(cc == ncc - 1))
                    gt = g_off[mi] + t
                    nc.scalar.activation(
                        g_sb[:pcnt, gt * 128:gt * 128 + 128], pg[:pcnt, :],
                        AF.Copy)

            for c in range(NVCH):
                ps = spp.tile([128, 512], F32, tag="ps")
                pairs_c = pairs[c]
                npc_c = len(pairs_c)
                half = (npc_c + 1) // 2
                wts = []
                for hb in range(2):
                    k0, k1 = hb * half, min((hb + 1) * half, npc_c)
                    wt = spw.tile([128, HNP, 512], BF16, tag="wsc")
                    if k1 > k0:
                        nc.sync.dma_start(
                            wt[:, :k1 - k0, :],
                            ap["wsc"].rearrange("(k p) h -> p k h", p=128)
                            [:, pair_off[c] + k0:pair_off[c] + k1, :])
                    wts.append(wt)
                for k, (mi, t) in enumerate(pairs_c):
                    HW = MAPS[mi][1] ** 2
                    pcnt = min(128, HW - t * 128)
                    gt = g_off[mi] + t
                    nc.tensor.matmul(
                        out=ps[:],
                        lhsT=g_sb[:pcnt, gt * 128:gt * 128 + 128],
                        rhs=wts[k // half][:pcnt, k % half, :],
                        start=(k == 0), stop=(k == npc_c - 1))
                nc.scalar.activation(xx[:, c * 512:(c + 1) * 512], ps[:],
                                     AF.Relu)
                emit_h1_rows(0, c * 4, 4, h1w0, spp)
                emit_h1_cols(0, c, spp)
                samp_done.append(c)

        # ---------------- phase 2: graph conv layers ----------------
        lp = ctx.enter_context(tc.tile_pool(name="msg", bufs=3))
        apb = ctx.enter_context(tc.tile_pool(name="apbuf", bufs=2))
        trp = ctx.enter_context(tc.tile_pool(name="trmsg", bufs=3))
        ohd = ctx.enter_context(tc.tile_pool(name="ohdma", bufs=2))
        psh = ctx.enter_context(tc.tile_pool(name="psh", bufs=2, space="PSUM"))
        pst = ctx.enter_context(tc.tile_pool(name="pst", bufs=2, space="PSUM"))
        psx = ctx.enter_context(tc.tile_pool(name="psx", bufs=2, space="PSUM"))

        pending = {}    # (l, g) -> (msg, ohg) or None
        pending_b = {}  # (l, g) -> (msg, s0, cutA, ng): deferred B segment

        H1W_CUT = (NT - 2 * GT) // 4  # h1 row writes covering tiles < SRC_CUT

        def _emit_gather_part(l, msg, s0, a, b, deps):
            gi = nc.gpsimd.dma_gather(
                out_ap=msg[:, a:b, :],
                in_ap=h1d2[l % 2][:],
                idxs_ap=srcw_t[:, (s0 + a) * 8:(s0 + b) * 8],
                num_idxs=(b - a) * 128,
                num_idxs_reg=(b - a) * 128,
                elem_size=HID,
                single_packet=False,
            )
            for wi in deps:
                tile.add_dep_helper(gi.ins, wi.ins,
                                    reason="h1 RAW: gather after write")

        def emit_gather(l, g, h1_writes, defer_b=False):
            """dma-gather + one-hot build for (layer l, group g). Split
            groups gather segment E (early sources) with a dependency on
            only the first H1W_CUT h1 writes; with defer_b the late-source
            segment is emitted later via emit_deferred_b."""
            s0, s1 = gsub_off[g], gsub_off[g + 1]
            ng = s1 - s0
            if ng == 0:
                pending[(l, g)] = None
                return
            msg = lp.tile([128, MAXSUB_G, 128], BF16, tag="msg")
            cutA = gmeta[g]["cutA"]
            if 0 < cutA < ng:
                _emit_gather_part(l, msg, s0, 0, cutA, h1_writes[:H1W_CUT])
                if defer_b:
                    pending_b[(l, g)] = (msg, s0, cutA, ng)
                else:
                    _emit_gather_part(l, msg, s0, cutA, ng, h1_writes)
            else:
                _emit_gather_part(l, msg, s0, 0, ng, h1_writes)
            noh = ng + gmeta[g]["nb"]
            b2 = gsub_off[g] + nb_off[g]
            ohg = ohd.tile([128, MAXOH_G, 128], FP8, tag="ohg")
            nc.vector.tensor_tensor(
                out=ohg[:, :noh, :],
                in0=dstloc_t[:, b2:b2 + noh, :].to_broadcast([128, noh, 128]),
                in1=iota_t[:].to_broadcast([128, noh, 128]),
                op=mybir.AluOpType.is_equal)
            pending[(l, g)] = (msg, ohg)

        ap_state = {}  # l -> per-layer ap-call emission state

        def _get_ap_state(l):
            if l not in ap_state:
                ap_state[l] = {
                    "next": [0] * NW,
                    "tr": [[None] * len(calls[w]) for w in range(NW)],
                    "flip": [0],
                }
            return ap_state[l]

        def emit_ap_call(l, w, k):
            st = _get_ap_state(l)
            bank = h1c_banks[l % 2]
            s0, s1, _t0, _t1, _gs, _ge = calls[w][k]
            ns = s1 - s0
            buf = apb.tile([128, MAXCALL * 128], F32, tag="apbuf")
            nc.gpsimd.ap_gather(
                out_ap=buf[:, :ns * 128],
                in_ap=bank[:, w * SUBW:(w + 1) * SUBW],
                idxs_ap=apw_t[w][:, s0 * 8:s1 * 8],
                channels=128, num_elems=SUBW, d=1, num_idxs=ns * 128)
            tr = trp.tile([128, MAXCALL, 128], BF16, tag="trmsg")
            st["tr"][w][k] = (tr, s0)
            for j4 in range(0, ns, 4):
                jn = min(4, ns - j4)
                pt = pst.tile([128, 512], F32, tag="pt")
                for j in range(jn):
                    nc.tensor.transpose(
                        pt[:, j * 128:(j + 1) * 128],
                        buf[:, (j4 + j) * 128:(j4 + j + 1) * 128],
                        ident_t[:])
                dst_sl = tr[:, j4:j4 + jn, :].rearrange("p s o -> p (s o)")
                if st["flip"][0] % 2 == 0:
                    nc.vector.tensor_copy(dst_sl, pt[:, :jn * 128])
                else:
                    nc.scalar.activation(dst_sl, pt[:, :jn * 128], AF.Copy)
                st["flip"][0] += 1

        def emit_eligible_calls(l, g):
            # round-robin across windows so group g's trmsg transposes
            # come before deeper-lookahead calls on the in-order engines
            st = _get_ap_state(l)
            while True:
                did = False
                for w in range(NW):
                    if (st["next"][w] < len(calls[w])
                            and calls[w][st["next"][w]][4] <= g + 2):
                        emit_ap_call(l, w, st["next"][w])
                        st["next"][w] += 1
                        did = True
                if not did:
                    break

        def _layer(l, h1_writes, last_layer):
            """Scatter groups for layer l; h1 for layer l+1 is emitted inside
            (pipelined). Returns layer l+1's h1_writes list."""
            h1_writes_next = []
            trmsg_tiles = _get_ap_state(l)["tr"]

            if (l, 0) in pending_b:
                msg_, s0_, cutA_, ng_ = pending_b.pop((l, 0))
                _emit_gather_part(l, msg_, s0_, cutA_, ng_, h1_writes)
            for g in range(min(PREFETCH, NGRP)):
                if (l, g) not in pending:
                    emit_gather(l, g, h1_writes)

            for g in range(NGRP):
                t_lo = g * GT
                t_hi = min((g + 1) * GT, NT)
                emit_eligible_calls(l, g)
                if g + PREFETCH < NGRP:
                    emit_gather(l, g + PREFETCH, h1_writes)

                W_ = (t_hi - t_lo) * 128
                px = psx.tile([128, GT * 128], F32, tag="px")

                got = pending.pop((l, g))
                if got is not None:
                    msg, ohg = got

                mms = []  # entries: (seg_id, kwargs)
                if l == 0:
                    vv2 = lpv.tile([3, GT * 128], BF16, tag="vt2")
                    nc.sync.dma_start(
                        vv2[:, :W_], ap["vertsT"][:, t_lo * 128:t_hi * 128])
                    for seg in range(0, W_, 512):
                        sw = min(512, W_ - seg)
                        c0 = t_lo * 128 + seg
                        mms.append((seg // 512,
                                    dict(out=px[:, seg:seg + sw],
                                         lhsT=g0m_t[:, 0:128],
                                         rhs=xx[:, c0:c0 + sw])))
                        mms.append((seg // 512,
                                    dict(out=px[:, seg:seg + sw],
                                         lhsT=g0v_t[:, 0:128],
                                         rhs=vv2[:, seg:seg + sw])))
                        mms.append((seg // 512,
                                    dict(out=px[:, seg:seg + sw],
                                         lhsT=erow_t[:, 0:128],
                                         rhs=ones_t[:, :sw])))
                else:
                    for seg in range(0, W_, 512):
                        sw = min(512, W_ - seg)
                        c0 = t_lo * 128 + seg
                        mms.append((seg // 512,
                                    dict(out=px[:, seg:seg + sw],
                                         lhsT=w0_t[:, (l - 1) * 128:l * 128],
                                         rhs=xx[:, c0:c0 + sw])))
                # DMA-half scatter (merged subchunks; one oh tile holds the
                # k0 columns [0, ng) and boundary columns [ng, ng+nb))
                if got is not None:
                    ng = gmeta[g]["ng"]
                    for (j, bloc, t_off) in gmeta[g]["mm"]:
                        ohc = j if bloc is None else ng + bloc
                        osl = slice(t_off * 128, (t_off + 1) * 128)
                        mms.append((t_off * 128 // 512,
                                    dict(out=px[:, osl], lhsT=msg[:, j, :],
                                         rhs=ohg[:, ohc, :])))
                # ap-half scatter
                for ti in range(t_hi - t_lo):
                    t = t_lo + ti
                    osl = slice(ti * 128, (ti + 1) * 128)
                    for w in range(NW):
                        na = n_ap[t][w]
                        if na == 0:
                            continue
                        kk = next(
                            i for i, c in enumerate(calls[w])
                            if c[2] <= t < c[3])
                        tr, trs0 = trmsg_tiles[w][kk]
                        for j in range(na):
                            s_loc = ap_off[t][w] - trs0 + j
                            s_ap = ap_off[t][w] + j + (0 if w == 0 else S_ap[0])
                            mms.append((ti * 128 // 512,
                                        dict(out=px[:, osl],
                                             lhsT=tr[:, s_loc, :],
                                             rhs=oh_ap[:, s_ap, :])))
                first_of = {}
                last_of = {}
                for i, (sg, _kw) in enumerate(mms):
                    first_of.setdefault(sg, i)
                    last_of[sg] = i
                for i, (sg, kw) in enumerate(mms):
                    nc.tensor.matmul(start=(first_of[sg] == i),
                                     stop=(last_of[sg] == i),
                                     skip_group_check=True, **kw)
                nc.scalar.activation(xx[:, t_lo * 128:t_hi * 128], px[:, :W_],
                                     AF.Relu)

                # ---- pipelined layer-(l+1) h1 production ----
                if not last_layer:
                    for c0 in range(t_lo, t_hi, 8):
                        emit_h1_rows(l + 1, c0, min(8, t_hi - c0),
                                     h1_writes_next, psh)
                    for c in chunks_by_group.get(g, []):
                        emit_h1_cols(l + 1, c, psh)
                else:
                    ost = lph.tile([128, GT * 3], F32, tag="ost")
                    for ti in range(t_hi - t_lo):
                        t = t_lo + ti
                        po = psh.tile([128, 512], F32, tag="ph")
                        nc.tensor.matmul(out=po[:, :3],
                                         lhsT=xx[:, t * 128:(t + 1) * 128],
                                         rhs=offw_t[:], start=True, stop=True)
                        nc.scalar.activation(ost[:, ti * 3:(ti + 1) * 3],
                                             po[:, :3], AF.Copy)
                    nc.sync.dma_start(
                        out.rearrange("(n p) c -> p n c", p=128)
                        [:, t_lo:t_hi, :],
                        ost[:, :(t_hi - t_lo) * 3]
                        .rearrange("p (n c) -> p n c", c=3))

            # cross-layer prefetch: the next layer's first ap calls go on
            # the Pool stream BEFORE the gathers (whose h1-write waits would
            # otherwise block them), then gather desc-gen for the first
            # groups so it overlaps this layer's tail.
            if not last_layer:
                emit_gather(l + 1, 0, h1_writes_next, defer_b=True)
                emit_gather(l + 1, 1, h1_writes_next)
            ap_state.pop(l, None)
            return h1_writes_next

        for _rep in range(repeat):
            h1w = h1w0
            pending.clear()
            for l in range(nlayers):
                h1w = _layer(l, h1w, l == nlayers - 1)

    nc.compile()
    return nc


_CACHE = {}


def kernel(**inputs) -> np.ndarray:
    cfg, per_core, post = _prep(inputs)
    key = (cfg["npair"], cfg["S_tot"], cfg["S_dma"], cfg["NB"],
           str(cfg["calls"]), str(cfg["gsub_off"]))
    if key not in _CACHE:
        _CACHE[key] = _build(cfg, per_core[0])
    nc = _CACHE[key]
    res = run_bass_kernel_spmd(nc, per_core, list(range(B)))
    outs = np.empty((B, V, 3), np.float32)
    for m in range(B):
        rows = res.results[m]["out"]
        outs[m] = rows[post["slot_of"][m][np.arange(V)]]
    return outs.reshape(B * V, 3)


if __name__ == "__main__":
    pass
